# revision 4
# baseline (speedup 1.0000x reference)
"""Trainium2 Bass kernel for nn_NeurosynapticEventEncoder.

Computes the reference model:
    sort events by timestamp -> event MLP + LN -> temporal MLP ->
    concat/amp-gate -> proj 2H->H + LN -> + sinusoidal pos enc ->
    Q = ctx@Wq+bq, Kb = ctx@Wk+bk, V = ctx@Wv+bv,
    K[i,j,h] = Kb[j,h] + 0.1*conn[i,j]  (conn = 1/(1+|i-j|))

Sharding: the huge output K [S,S,H] (536 MB fp32) is sharded over its
first axis (i) across 8 cores; everything upstream is tiny and computed
replicated on every core (no collectives). Q/V are taken from core 0.

Device layout: activations are kept transposed (X^T, [D, S]) so every
matmul uses the weight matrix as-stored for the stationary operand
(out = lhsT.T @ rhs).  LayerNorm over H (the partition dim in this
layout) uses PE ones-matmul column sums and PE rank-1 broadcasts.
The K expansion is per-partition-scalar adds (DVE tensor_scalar / ACT
Identity-bias) producing 8.4MB coalesced DMA writes.
"""

import math
from contextlib import ExitStack

import numpy as np

import concourse.bass as bass
import concourse.tile as tile
from concourse import bacc, mybir
from concourse.bass_utils import run_bass_kernel_spmd

S = 512
D_IN = 256
H = 512
N_CORES = 8
I_SH = S // N_CORES  # K rows per core = 64
IBLK = 8             # i-rows per K-write tile/DMA (8 * 4 * 512 * 4B = 8.4 MB per DMA)
F32 = mybir.dt.float32
AF = mybir.ActivationFunctionType
ALU = mybir.AluOpType
EPS = 1e-5


# ----------------------------------------------------------------------------
# device program
# ----------------------------------------------------------------------------

def _layer_norm(nc, pool, ps_red, ps_bc, x_ap, g_sb, b_sb, out_ap,
                ones_col, ones_row, eps_sb, post_scale_sb=None):
    """LN over the partition axis (4 x [128, 512] tiles = 512 rows of H).

    x_ap/out_ap: [128, 4, 512] SBUF APs (transposed layout [H, S]).
    g_sb/b_sb:   [128, 4] per-partition gain/bias.
    post_scale_sb: optional [128, 512] broadcast tile multiplied in after.
    """
    nkt = x_ap.shape[1]
    # column sums (over H) via ones-matmul -> [1, S]
    cs_ps = ps_red.tile([1, S], F32, name="cs_ps")
    for kt in range(nkt):
        nc.tensor.matmul(cs_ps, ones_col, x_ap[:, kt, :],
                         start=(kt == 0), stop=(kt == nkt - 1))
    csq_ps = ps_red.tile([1, S], F32, name="csq_ps")
    for kt in range(nkt):
        sq = pool.tile([128, S], F32, name="sq_scratch", bufs=2)
        nc.vector.tensor_mul(sq, x_ap[:, kt, :], x_ap[:, kt, :])
        nc.tensor.matmul(csq_ps, ones_col, sq,
                         start=(kt == 0), stop=(kt == nkt - 1))
    mean_r = pool.tile([1, S], F32, name="mean_r")
    nc.scalar.activation(mean_r, cs_ps, AF.Copy, bias=0.0, scale=1.0 / H)
    msq_r = pool.tile([1, S], F32, name="msq_r")
    nc.scalar.activation(msq_r, csq_ps, AF.Copy, bias=0.0, scale=1.0 / H)
    m2_r = pool.tile([1, S], F32, name="m2_r")
    nc.vector.tensor_mul(m2_r, mean_r, mean_r)
    var_r = pool.tile([1, S], F32, name="var_r")
    nc.vector.tensor_sub(var_r, msq_r, m2_r)
    std_r = pool.tile([1, S], F32, name="std_r")
    nc.scalar.activation(std_r, var_r, AF.Sqrt, bias=eps_sb[0:1, 0:1], scale=1.0)
    rstd_r = pool.tile([1, S], F32, name="rstd_r")
    nc.vector.reciprocal(rstd_r, std_r)
    mrs_r = pool.tile([1, S], F32, name="mrs_r")
    nc.vector.tensor_mul(mrs_r, mean_r, rstd_r)
    # broadcast rstd and mean*rstd across partitions via rank-1 matmul
    a_ps = ps_bc.tile([128, S], F32, name="a_ps")
    nc.tensor.matmul(a_ps, ones_row, rstd_r, start=True, stop=True)
    m_ps = ps_bc.tile([128, S], F32, name="m_ps")
    nc.tensor.matmul(m_ps, ones_row, mrs_r, start=True, stop=True)
    a_sb = pool.tile([128, S], F32, name="a_sb")
    nc.scalar.copy(a_sb, a_ps)
    m_sb = pool.tile([128, S], F32, name="m_sb")
    nc.scalar.copy(m_sb, m_ps)
    # apply: out = (x*rstd - mean*rstd) * g + b   (g, b per-partition)
    for kt in range(nkt):
        t1 = pool.tile([128, S], F32, name="ln_t1", bufs=2)
        nc.vector.tensor_mul(t1, x_ap[:, kt, :], a_sb)
        nc.vector.tensor_sub(t1, t1, m_sb)
        nc.vector.tensor_scalar(
            out=out_ap[:, kt, :], in0=t1,
            scalar1=g_sb[:, kt:kt + 1], scalar2=b_sb[:, kt:kt + 1],
            op0=ALU.mult, op1=ALU.add)
        if post_scale_sb is not None:
            nc.vector.tensor_mul(out_ap[:, kt, :], out_ap[:, kt, :], post_scale_sb)


def _build_nc():
    nc = bacc.Bacc()

    def inp(name, shape):
        return nc.declare_dram_parameter(name, list(shape), F32, isOutput=False)

    featsT_d = inp("featsT", (D_IN, S))
    ts_d = inp("ts_s", (S,))
    amps_d = inp("amps_s", (S,))
    We1_d = inp("We1", (D_IN, H // 2))
    be1_d = inp("be1", (H // 2,))
    We2_d = inp("We2", (H // 2, H))
    be2_d = inp("be2", (H,))
    Wt1_d = inp("Wt1", (1, H // 4))
    bt1_d = inp("bt1", (H // 4,))
    Wt2_d = inp("Wt2", (H // 4, H // 2))
    bt2_d = inp("bt2", (H // 2,))
    Wc_d = inp("Wc", (2 * H, H))
    bc_d = inp("bc", (H,))
    gev_d = inp("g_ev", (H,))
    bev_d = inp("b_ev", (H,))
    gsr_d = inp("g_sr", (H,))
    bsr_d = inp("b_sr", (H,))
    teT_d = inp("teT", (H, S))
    Wq_d = inp("Wq", (H, H))
    Wk_d = inp("Wk", (H, H))
    Wv_d = inp("Wv", (H, H))
    bq_d = inp("bq", (H,))
    bk_d = inp("bk", (H,))
    bv_d = inp("bv", (H,))
    C_d = inp("Csh", (S, I_SH))  # 0.1 * conn[:, i_shard]  as [j, i_local]

    Ksh_d = nc.declare_dram_parameter("K_sh", [I_SH, S, H], F32, isOutput=True)
    q_d = nc.declare_dram_parameter("q_out", [S, H], F32, isOutput=True)
    v_d = nc.declare_dram_parameter("v_out", [S, H], F32, isOutput=True)

    with tile.TileContext(nc) as tc:
        with ExitStack() as ctx:
            persist = ctx.enter_context(tc.tile_pool(name="persist", bufs=1))
            ps_mm = ctx.enter_context(
                tc.tile_pool(name="ps_mm", bufs=3, space=bass.MemorySpace.PSUM))
            ps_red = ctx.enter_context(
                tc.tile_pool(name="ps_red", bufs=1, space=bass.MemorySpace.PSUM))
            ps_bc = ctx.enter_context(
                tc.tile_pool(name="ps_bc", bufs=1, space=bass.MemorySpace.PSUM))

            # ---- persistent tiles ----
            ones_col = persist.tile([128, 1], F32)
            nc.vector.memset(ones_col, 1.0)
            ones_row = persist.tile([1, 128], F32)
            nc.vector.memset(ones_row, 1.0)
            eps_sb = persist.tile([1, 1], F32)
            nc.vector.memset(eps_sb, EPS)
            kb_sb = persist.tile([128, 4, S], F32)     # K base, [j, h] layout
            ctx_sb = persist.tile([128, 4, S], F32)    # ctx^T  [H, S]
            C_sb = persist.tile([128, 4, I_SH], F32)   # 0.1*conn, [j, i_local]
            nc.sync.dma_start(
                out=C_sb, in_=C_d[:].rearrange("(jt p) i -> p jt i", p=128))

            with tc.tile_pool(name="p1", bufs=1) as p1:
                # ---- load inputs (small/early-needed first) ----
                def load(name, shape, dram_ap, engine=None):
                    t = p1.tile(shape, F32, name=name)
                    (engine or nc.sync).dma_start(out=t, in_=dram_ap)
                    return t

                We1_sb = load("We1_sb", [128, 2, H // 2],
                              We1_d[:].rearrange("(kt p) m -> p kt m", p=128))
                be1_sb = load("be1_sb", [128, 2],
                              be1_d[:].rearrange("(t p) -> p t", p=128), nc.gpsimd)
                feats_sb = load("feats_sb", [128, 2, S],
                                featsT_d[:].rearrange("(kt p) s -> p kt s", p=128))
                ts_sb = load("ts_sb", [1, S], ts_d[:].unsqueeze(0))
                Wt1_sb = load("Wt1_sb", [1, H // 4], Wt1_d[:])
                bt1_sb = load("bt1_sb", [128, 1],
                              bt1_d[:].rearrange("(t p) -> p t", p=128), nc.gpsimd)
                Wt2_sb = load("Wt2_sb", [128, H // 2], Wt2_d[:])
                bt2_sb = load("bt2_sb", [128, 2],
                              bt2_d[:].rearrange("(t p) -> p t", p=128), nc.gpsimd)
                We2_sb = load("We2_sb", [128, 2, H],
                              We2_d[:].rearrange("(kt p) m -> p kt m", p=128))
                be2_sb = load("be2_sb", [128, 4],
                              be2_d[:].rearrange("(t p) -> p t", p=128), nc.gpsimd)
                gev_sb = load("gev_sb", [128, 4],
                              gev_d[:].rearrange("(t p) -> p t", p=128), nc.gpsimd)
                bev_sb = load("bev_sb", [128, 4],
                              bev_d[:].rearrange("(t p) -> p t", p=128), nc.gpsimd)
                amps_sb = load("amps_sb", [1, S], amps_d[:].unsqueeze(0))
                Wc_sb = load("Wc_sb", [128, 8, H],
                             Wc_d[:].rearrange("(kt p) m -> p kt m", p=128))
                bc_sb = load("bc_sb", [128, 4],
                             bc_d[:].rearrange("(t p) -> p t", p=128), nc.gpsimd)
                gsr_sb = load("gsr_sb", [128, 4],
                              gsr_d[:].rearrange("(t p) -> p t", p=128), nc.gpsimd)
                bsr_sb = load("bsr_sb", [128, 4],
                              bsr_d[:].rearrange("(t p) -> p t", p=128), nc.gpsimd)
                te_sb = load("te_sb", [128, 4, S],
                             teT_d[:].rearrange("(kt p) s -> p kt s", p=128))
                Wk_sb = load("Wk_sb", [128, 4, H],
                             Wk_d[:].rearrange("(kt p) m -> p kt m", p=128))
                bk_sb = load("bk_sb", [1, H], bk_d[:].unsqueeze(0))
                Wq_sb = load("Wq_sb", [128, 4, H],
                             Wq_d[:].rearrange("(kt p) m -> p kt m", p=128))
                bq_sb = load("bq_sb", [1, H], bq_d[:].unsqueeze(0))
                Wv_sb = load("Wv_sb", [128, 4, H],
                             Wv_d[:].rearrange("(kt p) m -> p kt m", p=128))
                bv_sb = load("bv_sb", [1, H], bv_d[:].unsqueeze(0))

                # ---- event MLP: h1 = relu(feats@We1+be1), h2 = h1@We2+be2 ----
                h1_sb = p1.tile([128, 2, S], F32)  # [H/2, S]
                for mt in range(2):
                    mm_ps = ps_mm.tile([128, S], F32, name="mm_ps")
                    for kt in range(2):
                        nc.tensor.matmul(
                            mm_ps, We1_sb[:, kt, bass.ts(mt, 128)],
                            feats_sb[:, kt, :], start=(kt == 0), stop=(kt == 1))
                    nc.scalar.activation(h1_sb[:, mt, :], mm_ps, AF.Relu,
                                         bias=be1_sb[:, mt:mt + 1], scale=1.0)
                h2_sb = p1.tile([128, 4, S], F32)  # [H, S]
                for mt in range(4):
                    mm_ps = ps_mm.tile([128, S], F32, name="mm_ps")
                    for kt in range(2):
                        nc.tensor.matmul(
                            mm_ps, We2_sb[:, kt, bass.ts(mt, 128)],
                            h1_sb[:, kt, :], start=(kt == 0), stop=(kt == 1))
                    nc.scalar.activation(h2_sb[:, mt, :], mm_ps, AF.Identity,
                                         bias=be2_sb[:, mt:mt + 1], scale=1.0)

                # ---- temporal MLP: t = relu(ts@Wt1+bt1)@Wt2+bt2 ----
                t1_ps = ps_mm.tile([128, S], F32, name="mm_ps")
                nc.tensor.matmul(t1_ps, Wt1_sb, ts_sb, start=True, stop=True)
                t1_sb = p1.tile([128, S], F32)
                nc.scalar.activation(t1_sb, t1_ps, AF.Relu,
                                     bias=bt1_sb[:, 0:1], scale=1.0)
                t2_sb = p1.tile([128, 2, S], F32)  # [H/2, S]
                for mt in range(2):
                    mm_ps = ps_mm.tile([128, S], F32, name="mm_ps")
                    nc.tensor.matmul(mm_ps, Wt2_sb[:, bass.ts(mt, 128)], t1_sb,
                                     start=True, stop=True)
                    nc.scalar.activation(t2_sb[:, mt, :], mm_ps, AF.Identity,
                                         bias=bt2_sb[:, mt:mt + 1], scale=1.0)

                # ---- amplitude broadcast [128, S] ----
                amps_ps = ps_bc.tile([128, S], F32, name="a_ps")
                nc.tensor.matmul(amps_ps, ones_row, amps_sb, start=True, stop=True)
                ampsB_sb = p1.tile([128, S], F32)
                nc.scalar.copy(ampsB_sb, amps_ps)

                # ---- LN1 (g_ev/b_ev) with amp gating folded in ----
                c1_sb = p1.tile([128, 4, S], F32)  # amps * LN(h2)
                _layer_norm(nc, p1, ps_red, ps_bc, h2_sb, gev_sb, bev_sb, c1_sb,
                            ones_col, ones_row, eps_sb, post_scale_sb=ampsB_sb)
                tts_sb = p1.tile([128, 2, S], F32)  # amps * t
                for mt in range(2):
                    nc.vector.tensor_mul(tts_sb[:, mt, :], t2_sb[:, mt, :], ampsB_sb)

                # ---- comb @ Wc + bc  (comb^T ktiles: c1 x4, tts x2, tts x2) ----
                comb_kt = [c1_sb[:, 0, :], c1_sb[:, 1, :], c1_sb[:, 2, :],
                           c1_sb[:, 3, :], tts_sb[:, 0, :], tts_sb[:, 1, :],
                           tts_sb[:, 0, :], tts_sb[:, 1, :]]
                post_sb = p1.tile([128, 4, S], F32)
                for mt in range(4):
                    mm_ps = ps_mm.tile([128, S], F32, name="mm_ps")
                    for kt in range(8):
                        nc.tensor.matmul(
                            mm_ps, Wc_sb[:, kt, bass.ts(mt, 128)], comb_kt[kt],
                            start=(kt == 0), stop=(kt == 7))
                    nc.scalar.activation(post_sb[:, mt, :], mm_ps, AF.Identity,
                                         bias=bc_sb[:, mt:mt + 1], scale=1.0)

                # ---- LN2 (g_sr/b_sr), then + te -> ctx^T ----
                ln2_sb = p1.tile([128, 4, S], F32)
                _layer_norm(nc, p1, ps_red, ps_bc, post_sb, gsr_sb, bsr_sb,
                            ln2_sb, ones_col, ones_row, eps_sb)
                for kt in range(4):
                    nc.vector.tensor_add(ctx_sb[:, kt, :], ln2_sb[:, kt, :],
                                         te_sb[:, kt, :])

                # ---- projections: out[s,h] = ctx@W + b  (lhsT = ctx^T) ----
                def project(W_sb, b_row, out_sb):
                    for st in range(4):
                        mm_ps = ps_mm.tile([128, H], F32, name="mm_ps")
                        for kt in range(4):
                            nc.tensor.matmul(
                                mm_ps, ctx_sb[:, kt, bass.ts(st, 128)],
                                W_sb[:, kt, :], start=(kt == 0), stop=False)
                        nc.tensor.matmul(mm_ps, ones_row, b_row,
                                         start=False, stop=True)
                        nc.scalar.copy(out_sb[:, st, :], mm_ps)

                project(Wk_sb, bk_sb, kb_sb)  # K base first: K-write depends on it
                q_sb = p1.tile([128, 4, H], F32)
                project(Wq_sb, bq_sb, q_sb)
                nc.sync.dma_start(
                    out=q_d[:].rearrange("(st p) h -> p st h", p=128), in_=q_sb)
                v_sb = p1.tile([128, 4, H], F32)
                project(Wv_sb, bv_sb, v_sb)
                nc.sync.dma_start(
                    out=v_d[:].rearrange("(st p) h -> p st h", p=128), in_=v_sb)

            # ---- K expansion: K[i, jt*128+p, h] = kb[p, jt, h] + C[p, jt, i] ----
            kw = ctx.enter_context(tc.tile_pool(name="kw", bufs=2))
            Kv = Ksh_d[:].rearrange("i (jt p) h -> p i jt h", p=128)
            nslab = 0
            for b0 in range(0, I_SH, IBLK):
                T = kw.tile([128, IBLK, 4, H], F32, name="kT")
                for ii in range(IBLK):
                    il = b0 + ii
                    for jt in range(4):
                        slab = T[:, ii, jt, :]
                        if nslab % 3 == 2:
                            nc.scalar.add(slab, kb_sb[:, jt, :],
                                          C_sb[:, jt, il:il + 1])
                        else:
                            nc.vector.tensor_scalar_add(slab, kb_sb[:, jt, :],
                                                        C_sb[:, jt, il:il + 1])
                        nslab += 1
                nc.sync.dma_start(out=Kv[:, b0:b0 + IBLK, :, :], in_=T)

    nc.finalize()  # Bacc: runs wait-splitting + register allocation passes
    return nc


_NC = None


def _get_nc():
    global _NC
    if _NC is None:
        _NC = _build_nc()
    return _NC


# ----------------------------------------------------------------------------
# host side
# ----------------------------------------------------------------------------

def _host_constants():
    pos = np.arange(S, dtype=np.float32)[:, None]
    div = np.exp(np.arange(0, H, 2, dtype=np.float32)
                 * np.float32(-math.log(10000.0) / H))
    ang = pos * div[None, :]
    te = np.stack([np.sin(ang), np.cos(ang)], axis=-1).reshape(S, H)
    teT = np.ascontiguousarray(te.T.astype(np.float32))
    idx = np.arange(S)
    conn = (1.0 / (1.0 + np.abs(idx[:, None] - idx[None, :]).astype(np.float32)))
    C = (0.1 * conn).astype(np.float32)
    return teT, C


def _f32(x):
    return np.ascontiguousarray(np.asarray(x, dtype=np.float32))


def build_in_maps(timestamps, features, amplitudes,
                  We1, be1, We2, be2, g_ev, b_ev,
                  Wt1, bt1, Wt2, bt2, Wc, bc, g_sr, b_sr,
                  Wq, bq, Wk, bk, Wv, bv):
    timestamps = _f32(timestamps)
    order = np.argsort(timestamps, kind="stable")
    ts_s = timestamps[order]
    featsT = np.ascontiguousarray(_f32(features)[order].T)
    amps_s = _f32(amplitudes)[order]
    teT, C = _host_constants()

    common = {
        "featsT": featsT, "ts_s": ts_s, "amps_s": amps_s,
        "We1": _f32(We1), "be1": _f32(be1), "We2": _f32(We2), "be2": _f32(be2),
        "Wt1": _f32(Wt1), "bt1": _f32(bt1), "Wt2": _f32(Wt2), "bt2": _f32(bt2),
        "Wc": _f32(Wc), "bc": _f32(bc),
        "g_ev": _f32(g_ev), "b_ev": _f32(b_ev),
        "g_sr": _f32(g_sr), "b_sr": _f32(b_sr),
        "teT": teT,
        "Wq": _f32(Wq), "bq": _f32(bq), "Wk": _f32(Wk), "bk": _f32(bk),
        "Wv": _f32(Wv), "bv": _f32(bv),
    }
    in_maps = []
    for c in range(N_CORES):
        m = dict(common)
        m["Csh"] = np.ascontiguousarray(C[:, c * I_SH:(c + 1) * I_SH])
        in_maps.append(m)
    return in_maps


def kernel(timestamps, features, amplitudes, neuron_ids,
           We1, be1, We2, be2, g_ev, b_ev,
           Wt1, bt1, Wt2, bt2, Wc, bc, g_sr, b_sr,
           Wq, bq, Wk, bk, Wv, bv, **_unused):
    del neuron_ids  # unused by the reference model
    nc = _get_nc()
    in_maps = build_in_maps(
        timestamps, features, amplitudes,
        We1, be1, We2, be2, g_ev, b_ev,
        Wt1, bt1, Wt2, bt2, Wc, bc, g_sr, b_sr,
        Wq, bq, Wk, bk, Wv, bv)
    res = run_bass_kernel_spmd(nc, in_maps, core_ids=list(range(N_CORES)))
    outs = res.results
    K = np.concatenate([outs[c]["K_sh"] for c in range(N_CORES)], axis=0)
    Q = outs[0]["q_out"].reshape(1, S, H)
    V = outs[0]["v_out"].reshape(1, S, H)
    return Q, K, V


# revision 15
# speedup vs baseline: 1.3279x; 1.3279x over previous
"""Trainium2 Bass kernel for nn_NeurosynapticEventEncoder.

Reference model:
    sort events by timestamp -> event MLP + LN -> temporal MLP ->
    concat/amp-gate -> proj 2H->H + LN -> + sinusoidal pos enc -> ctx
    Q = ctx@Wq+bq, Kb = ctx@Wk+bk, V = ctx@Wv+bv,
    K[i,j,h] = Kb[j,h] + 0.1*conn[i,j]  (conn = 1/(1+|i-j|))

Sharding: K [S,S,H] (536 MB fp32) is sharded over its SECOND axis (j)
across 8 cores.  Everything upstream of K is row-wise in s (LayerNorms
are per-row, matmuls are row-independent), so each core computes only
its 64-row slice of the whole chain — ctx/Kb/Q/V shards — with no
collectives and no redundant compute.  Host concatenates Q/V (axis 0)
and K (axis 1).

Device layout: the MLP chain keeps activations transposed (X^T, [D, s])
so matmuls use weight matrices as-stored for the stationary operand
(out = lhsT.T @ rhs).  The 2H->H projection flips orientation
(lhsT = comb^T) to produce `post` as [s, H], which makes LayerNorm 2 a
native free-dim bn_stats and lets the amplitude gate ride the PSUM->SBUF
copy as a per-partition scale.  ctx is then transposed once via the PE.
K expansion: K_flat[(i,jl), h] = kb[jl, h] + 0.1*conn[i, j0+jl]; each
[128, 512] slab covers two i values (kb duplicated into both partition
halves), written out as 4 MB contiguous DMAs.

Host-side prep (index/constant work only): argsort by timestamps and
gather, transposed feature shard, sinusoidal table, conn table, and the
exact weight folds Wc_eff = [Wc[:512]; Wc[512:768]+Wc[768:]] (the two
tiled t blocks see identical activations) and te' = te + b_sr.
"""

import math
from contextlib import ExitStack

import numpy as np

import concourse.bass as bass
import concourse.tile as tile
from concourse import bacc, mybir
from concourse.bass_utils import run_bass_kernel_spmd
from concourse.masks import make_identity

S = 512
D_IN = 256
H = 512
N_CORES = 8
J_SH = S // N_CORES       # 64 columns of K / rows of ctx per core
NBLK = S * J_SH // 128    # 256 slabs of [128, 512] per core
BLK_PER_TILE = 16         # 16 slabs -> 4 MB per K-write DMA
F32 = mybir.dt.float32
AF = mybir.ActivationFunctionType
ALU = mybir.AluOpType
EPS = 1e-5


# ----------------------------------------------------------------------------
# device program
# ----------------------------------------------------------------------------

def _layer_norm_part(nc, pool, ps_red, ps_bc, x_ap, g_sb, b_sb, out_ap,
                     ones_col, ones_row, eps_sb):
    """LN over the partition axis (nkt x [128, n] tiles), transposed layout."""
    nkt, n = x_ap.shape[1], x_ap.shape[2]
    cs_ps = ps_red.tile([1, n], F32, name="cs_ps")
    for kt in range(nkt):
        nc.tensor.matmul(cs_ps, ones_col, x_ap[:, kt, :],
                         start=(kt == 0), stop=(kt == nkt - 1))
    csq_ps = ps_red.tile([1, n], F32, name="csq_ps")
    for kt in range(nkt):
        sq = pool.tile([128, n], F32, name="sq_scratch", bufs=2)
        nc.vector.tensor_mul(sq, x_ap[:, kt, :], x_ap[:, kt, :])
        nc.tensor.matmul(csq_ps, ones_col, sq,
                         start=(kt == 0), stop=(kt == nkt - 1))
    mean_r = pool.tile([1, n], F32, name="mean_r")
    nc.scalar.activation(mean_r, cs_ps, AF.Copy, bias=0.0, scale=1.0 / H)
    msq_r = pool.tile([1, n], F32, name="msq_r")
    nc.scalar.activation(msq_r, csq_ps, AF.Copy, bias=0.0, scale=1.0 / H)
    m2_r = pool.tile([1, n], F32, name="m2_r")
    nc.vector.tensor_mul(m2_r, mean_r, mean_r)
    var_r = pool.tile([1, n], F32, name="var_r")
    nc.vector.tensor_sub(var_r, msq_r, m2_r)
    std_r = pool.tile([1, n], F32, name="std_r")
    nc.scalar.activation(std_r, var_r, AF.Sqrt, bias=eps_sb[0:1, 0:1], scale=1.0)
    rstd_r = pool.tile([1, n], F32, name="rstd_r")
    nc.vector.reciprocal(rstd_r, std_r)
    mrs_r = pool.tile([1, n], F32, name="mrs_r")
    nc.vector.tensor_mul(mrs_r, mean_r, rstd_r)
    a_ps = ps_bc.tile([128, n], F32, name="a_ps", tag="bc_ps")
    nc.tensor.matmul(a_ps, ones_row, rstd_r, start=True, stop=True)
    m_ps = ps_bc.tile([128, n], F32, name="m_ps", tag="bc_ps")
    nc.tensor.matmul(m_ps, ones_row, mrs_r, start=True, stop=True)
    a_sb = pool.tile([128, n], F32, name="a_sb")
    nc.scalar.copy(a_sb, a_ps)
    m_sb = pool.tile([128, n], F32, name="m_sb")
    nc.scalar.copy(m_sb, m_ps)
    for kt in range(nkt):
        t1 = pool.tile([128, n], F32, name="ln_t1", bufs=2)
        nc.vector.tensor_mul(t1, x_ap[:, kt, :], a_sb)
        nc.vector.tensor_sub(t1, t1, m_sb)
        nc.vector.tensor_scalar(
            out=out_ap[:, kt, :], in0=t1,
            scalar1=g_sb[:, kt:kt + 1], scalar2=b_sb[:, kt:kt + 1],
            op0=ALU.mult, op1=ALU.add)


def _build_nc():
    nc = bacc.Bacc()
    n = J_SH  # per-core s extent

    def inp(name, shape):
        return nc.declare_dram_parameter(name, list(shape), F32, isOutput=False)

    # per-core shards
    featsT_d = inp("featsT_sh", (D_IN, n))
    ts_d = inp("ts_sh", (n,))
    amps_d = inp("amps_sh", (n,))
    te_d = inp("te_sh", (n, H))          # te[s_shard] + b_sr
    C_d = inp("conn2_sh", (128, NBLK))   # 0.1*conn arranged per slab
    # replicated weights
    We1_d = inp("We1", (D_IN, H // 2))
    be1_d = inp("be1", (H // 2,))
    We2_d = inp("We2", (H // 2, H))
    be2_d = inp("be2", (H,))
    Wt1_d = inp("Wt1", (1, H // 4))
    bt1_d = inp("bt1", (H // 4,))
    Wt2_d = inp("Wt2", (H // 4, H // 2))
    bt2_d = inp("bt2", (H // 2,))
    Wc_d = inp("Wc_eff", (H + H // 2, H))
    bc_d = inp("bc", (H,))
    gev_d = inp("g_ev", (H,))
    bev_d = inp("b_ev", (H,))
    gsr_d = inp("g_sr", (H,))
    Wq_d = inp("Wq", (H, H))
    Wk_d = inp("Wk", (H, H))
    Wv_d = inp("Wv", (H, H))
    bq_d = inp("bq", (H,))
    bk_d = inp("bk", (H,))
    bv_d = inp("bv", (H,))

    Ksh_d = nc.declare_dram_parameter("K_sh", [S, J_SH, H], F32, isOutput=True)
    q_d = nc.declare_dram_parameter("q_out", [n, H], F32, isOutput=True)
    v_d = nc.declare_dram_parameter("v_out", [n, H], F32, isOutput=True)

    with tile.TileContext(nc) as tc:
        with ExitStack() as ctx:
            persist = ctx.enter_context(tc.tile_pool(name="persist", bufs=1))
            ps_mm = ctx.enter_context(
                tc.tile_pool(name="ps_mm", bufs=2, space=bass.MemorySpace.PSUM))
            ps_red = ctx.enter_context(
                tc.tile_pool(name="ps_red", bufs=1, space=bass.MemorySpace.PSUM))
            ps_bc = ctx.enter_context(
                tc.tile_pool(name="ps_bc", bufs=2, space=bass.MemorySpace.PSUM))

            ones_col = persist.tile([128, 1], F32)
            nc.vector.memset(ones_col, 1.0)
            ones_row = persist.tile([1, 128], F32)
            nc.vector.memset(ones_row, 1.0)
            eps_sb = persist.tile([128, 1], F32)
            nc.vector.memset(eps_sb, EPS)
            ident = persist.tile([128, 128], F32)
            make_identity(nc, ident)
            kb2_sb = persist.tile([128, H], F32)    # kb duplicated in both halves
            ctxT_sb = persist.tile([128, 4, n], F32)
            C_sb = persist.tile([128, NBLK], F32)
            nc.sync.dma_start(out=C_sb, in_=C_d[:])

            with tc.tile_pool(name="p1", bufs=1) as p1:
                def load(name, shape, dram_ap, engine=None):
                    t = p1.tile(shape, F32, name=name)
                    (engine or nc.sync).dma_start(out=t, in_=dram_ap)
                    return t

                We1_sb = load("We1_sb", [128, 2, H // 2],
                              We1_d[:].rearrange("(kt p) m -> p kt m", p=128))
                be1_sb = load("be1_sb", [128, 2],
                              be1_d[:].rearrange("(t p) -> p t", p=128), nc.gpsimd)
                feats_sb = load("feats_sb", [128, 2, n],
                                featsT_d[:].rearrange("(kt p) s -> p kt s", p=128))
                ts_sb = load("ts_sb", [1, n], ts_d[:].unsqueeze(0))
                amps_sb = load("amps_sb", [n, 1], amps_d[:].unsqueeze(-1))
                Wt1_sb = load("Wt1_sb", [1, H // 4], Wt1_d[:])
                bt1_sb = load("bt1_sb", [128, 1],
                              bt1_d[:].rearrange("(t p) -> p t", p=128), nc.gpsimd)
                Wt2_sb = load("Wt2_sb", [128, H // 2], Wt2_d[:])
                bt2_sb = load("bt2_sb", [128, 2],
                              bt2_d[:].rearrange("(t p) -> p t", p=128), nc.gpsimd)
                We2_sb = load("We2_sb", [128, 2, H],
                              We2_d[:].rearrange("(kt p) m -> p kt m", p=128))
                be2_sb = load("be2_sb", [128, 4],
                              be2_d[:].rearrange("(t p) -> p t", p=128), nc.gpsimd)
                gev_sb = load("gev_sb", [128, 4],
                              gev_d[:].rearrange("(t p) -> p t", p=128), nc.gpsimd)
                bev_sb = load("bev_sb", [128, 4],
                              bev_d[:].rearrange("(t p) -> p t", p=128), nc.gpsimd)
                Wc_sb = load("Wc_sb", [128, 6, H],
                             Wc_d[:].rearrange("(kt p) m -> p kt m", p=128))
                bc_sb = load("bc_sb", [1, H], bc_d[:].unsqueeze(0))
                gsr_sb = load("gsr_sb", [1, H], gsr_d[:].unsqueeze(0))
                te_sb = load("te_sb", [n, H], te_d[:])
                Wk_sb = load("Wk_sb", [128, 4, H],
                             Wk_d[:].rearrange("(kt p) m -> p kt m", p=128))
                bk_sb = load("bk_sb", [1, H], bk_d[:].unsqueeze(0))
                Wq_sb = load("Wq_sb", [128, 4, H],
                             Wq_d[:].rearrange("(kt p) m -> p kt m", p=128))
                bq_sb = load("bq_sb", [1, H], bq_d[:].unsqueeze(0))
                Wv_sb = load("Wv_sb", [128, 4, H],
                             Wv_d[:].rearrange("(kt p) m -> p kt m", p=128))
                bv_sb = load("bv_sb", [1, H], bv_d[:].unsqueeze(0))

                # ---- event MLP (transposed: [D, s]) ----
                h1_sb = p1.tile([128, 2, n], F32)
                for mt in range(2):
                    mm_ps = ps_mm.tile([128, n], F32, name="mm_sm")
                    for kt in range(2):
                        nc.tensor.matmul(
                            mm_ps, We1_sb[:, kt, bass.ts(mt, 128)],
                            feats_sb[:, kt, :], start=(kt == 0), stop=(kt == 1))
                    nc.scalar.activation(h1_sb[:, mt, :], mm_ps, AF.Relu,
                                         bias=be1_sb[:, mt:mt + 1], scale=1.0)
                h2_sb = p1.tile([128, 4, n], F32)
                for mt in range(4):
                    mm_ps = ps_mm.tile([128, n], F32, name="mm_sm")
                    for kt in range(2):
                        nc.tensor.matmul(
                            mm_ps, We2_sb[:, kt, bass.ts(mt, 128)],
                            h1_sb[:, kt, :], start=(kt == 0), stop=(kt == 1))
                    nc.scalar.activation(h2_sb[:, mt, :], mm_ps, AF.Identity,
                                         bias=be2_sb[:, mt:mt + 1], scale=1.0)

                # ---- temporal MLP ----
                t1_ps = ps_mm.tile([128, n], F32, name="mm_sm")
                nc.tensor.matmul(t1_ps, Wt1_sb, ts_sb, start=True, stop=True)
                t1_sb = p1.tile([128, n], F32)
                nc.scalar.activation(t1_sb, t1_ps, AF.Relu,
                                     bias=bt1_sb[:, 0:1], scale=1.0)
                t2_sb = p1.tile([128, 2, n], F32)
                for mt in range(2):
                    mm_ps = ps_mm.tile([128, n], F32, name="mm_sm")
                    nc.tensor.matmul(mm_ps, Wt2_sb[:, bass.ts(mt, 128)], t1_sb,
                                     start=True, stop=True)
                    nc.scalar.activation(t2_sb[:, mt, :], mm_ps, AF.Identity,
                                         bias=bt2_sb[:, mt:mt + 1], scale=1.0)

                # ---- LN1 (over H = partition axis) ----
                c1_sb = p1.tile([128, 4, n], F32)
                _layer_norm_part(nc, p1, ps_red, ps_bc, h2_sb, gev_sb, bev_sb,
                                 c1_sb, ones_col, ones_row, eps_sb)

                # ---- comb @ Wc_eff + bc, then amp-gate on the copy ----
                comb_kt = [c1_sb[:, 0, :], c1_sb[:, 1, :], c1_sb[:, 2, :],
                           c1_sb[:, 3, :], t2_sb[:, 0, :], t2_sb[:, 1, :]]
                post_ps = ps_mm.tile([n, H], F32, name="mm_big")
                for kt in range(6):
                    nc.tensor.matmul(post_ps, comb_kt[kt], Wc_sb[:, kt, :],
                                     start=(kt == 0), stop=False)
                nc.tensor.matmul(post_ps, ones_row[:, 0:n], bc_sb,
                                 start=False, stop=True)
                post_sb = p1.tile([n, H], F32)
                nc.scalar.activation(post_sb, post_ps, AF.Copy,
                                     bias=0.0, scale=amps_sb)

                # ---- LN2 (over H = free axis, bn_stats) + g_sr + (te + b_sr) --
                stats = p1.tile([n, nc.vector.BN_STATS_DIM], F32)
                nc.vector.bn_stats(out=stats, in_=post_sb)
                mv = p1.tile([n, nc.vector.BN_AGGR_DIM], F32)
                nc.vector.bn_aggr(out=mv, in_=stats)
                std_c = p1.tile([n, 1], F32)
                nc.scalar.activation(std_c, mv[:, 1:2], AF.Sqrt,
                                     bias=eps_sb[0:n, 0:1], scale=1.0)
                rstd_c = p1.tile([n, 1], F32)
                nc.vector.reciprocal(rstd_c, std_c)
                xn_sb = p1.tile([n, H], F32)
                nc.vector.tensor_scalar(out=xn_sb, in0=post_sb,
                                        scalar1=mv[:, 0:1], scalar2=rstd_c,
                                        op0=ALU.subtract, op1=ALU.mult)
                g_ps = ps_mm.tile([n, H], F32, name="g_ps", tag="mm_big")
                nc.tensor.matmul(g_ps, ones_row[:, 0:n], gsr_sb,
                                 start=True, stop=True)
                g_bc = p1.tile([n, H], F32)
                nc.scalar.copy(g_bc, g_ps)
                ctx_sb = p1.tile([n, H], F32)
                nc.vector.tensor_mul(ctx_sb, xn_sb, g_bc)
                nc.vector.tensor_add(ctx_sb, ctx_sb, te_sb)

                # ---- transpose ctx -> ctx^T [H, s] ----
                for ht in range(4):
                    tp_ps = ps_bc.tile([128, n], F32, name="tp_ps", tag="bc_ps")
                    nc.tensor.transpose(tp_ps, ctx_sb[:, bass.ts(ht, 128)],
                                        ident[0:n, 0:n])
                    nc.scalar.copy(ctxT_sb[:, ht, :], tp_ps)

                # ---- projections: out[s_local, h] = ctx@W + b ----
                def project(W_sb, b_row, out_sb):
                    mm_ps = ps_mm.tile([n, H], F32, name="mm_big")
                    for kt in range(4):
                        nc.tensor.matmul(mm_ps, ctxT_sb[:, kt, :], W_sb[:, kt, :],
                                         start=(kt == 0), stop=False)
                    nc.tensor.matmul(mm_ps, ones_row[:, 0:n], b_row,
                                     start=False, stop=True)
                    nc.scalar.copy(out_sb, mm_ps)

                project(Wk_sb, bk_sb, kb2_sb[0:n, :])
                # duplicate kb into partitions 64..127 (slabs cover 2 i's)
                nc.gpsimd.dma_start(out=kb2_sb[n:2 * n, :], in_=kb2_sb[0:n, :])

                q_sb = p1.tile([n, H], F32)
                project(Wq_sb, bq_sb, q_sb)
                nc.sync.dma_start(out=q_d[:], in_=q_sb)
                v_sb = p1.tile([n, H], F32)
                project(Wv_sb, bv_sb, v_sb)
                nc.sync.dma_start(out=v_d[:], in_=v_sb)

            # ---- K expansion ----
            # K_sh[i, jl, h]; flat row i*64+jl; partition p=(i%2)*64+jl.
            kw = ctx.enter_context(tc.tile_pool(name="kw", bufs=3))
            Kv = Ksh_d[:].rearrange("(blk a) j h -> (a j) blk h", a=2)
            nslab = 0
            for b0 in range(0, NBLK, BLK_PER_TILE):
                kT = kw.tile([128, BLK_PER_TILE, H], F32, name="kT")
                for m in range(BLK_PER_TILE):
                    blk = b0 + m
                    slab = kT[:, m, :]
                    if nslab % 5 < 3:
                        nc.vector.tensor_scalar_add(slab, kb2_sb,
                                                    C_sb[:, blk:blk + 1])
                    else:
                        nc.scalar.add(slab, kb2_sb, C_sb[:, blk:blk + 1])
                    nslab += 1
                nc.sync.dma_start(out=Kv[:, b0:b0 + BLK_PER_TILE, :], in_=kT)

    nc.finalize()  # Bacc: wait-splitting + register allocation passes
    return nc


_NC = None


def _get_nc():
    global _NC
    if _NC is None:
        _NC = _build_nc()
    return _NC


# ----------------------------------------------------------------------------
# host side
# ----------------------------------------------------------------------------

def _host_constants():
    pos = np.arange(S, dtype=np.float32)[:, None]
    div = np.exp(np.arange(0, H, 2, dtype=np.float32)
                 * np.float32(-math.log(10000.0) / H))
    ang = pos * div[None, :]
    te = np.stack([np.sin(ang), np.cos(ang)], axis=-1).reshape(S, H)
    te = te.astype(np.float32)
    idx = np.arange(S)
    conn = (1.0 / (1.0 + np.abs(idx[:, None] - idx[None, :]).astype(np.float32)))
    C = (0.1 * conn).astype(np.float32)
    return te, C


def _f32(x):
    return np.ascontiguousarray(np.asarray(x, dtype=np.float32))


def build_in_maps(timestamps, features, amplitudes,
                  We1, be1, We2, be2, g_ev, b_ev,
                  Wt1, bt1, Wt2, bt2, Wc, bc, g_sr, b_sr,
                  Wq, bq, Wk, bk, Wv, bv):
    timestamps = _f32(timestamps)
    order = np.argsort(timestamps, kind="stable")
    ts_s = timestamps[order]
    featsT = np.ascontiguousarray(_f32(features)[order].T)  # [D_IN, S]
    amps_s = _f32(amplitudes)[order]
    te, C = _host_constants()
    b_sr = _f32(b_sr)
    Wc = _f32(Wc)
    # exact folds: the two tiled t blocks see identical activations
    Wc_eff = np.ascontiguousarray(
        np.concatenate([Wc[:H], Wc[H:H + H // 2] + Wc[H + H // 2:]], axis=0))
    te_b = te + b_sr[None, :]  # fold LN2 shift into the te add

    common = {
        "We1": _f32(We1), "be1": _f32(be1), "We2": _f32(We2), "be2": _f32(be2),
        "Wt1": _f32(Wt1), "bt1": _f32(bt1), "Wt2": _f32(Wt2), "bt2": _f32(bt2),
        "Wc_eff": Wc_eff, "bc": _f32(bc),
        "g_ev": _f32(g_ev), "b_ev": _f32(b_ev), "g_sr": _f32(g_sr),
        "Wq": _f32(Wq), "bq": _f32(bq), "Wk": _f32(Wk), "bk": _f32(bk),
        "Wv": _f32(Wv), "bv": _f32(bv),
    }
    in_maps = []
    for c in range(N_CORES):
        sh = slice(c * J_SH, (c + 1) * J_SH)
        # conn2[p=(i%2)*64+jl, blk=i//2] = 0.1*conn[i, c*64+jl]
        base = C[:, sh]                       # [i=512, jl=64]
        conn2 = np.ascontiguousarray(
            base.reshape(NBLK, 2, J_SH).transpose(1, 2, 0).reshape(128, NBLK))
        m = dict(common)
        m["featsT_sh"] = np.ascontiguousarray(featsT[:, sh])
        m["ts_sh"] = np.ascontiguousarray(ts_s[sh])
        m["amps_sh"] = np.ascontiguousarray(amps_s[sh])
        m["te_sh"] = np.ascontiguousarray(te_b[sh])
        m["conn2_sh"] = conn2
        in_maps.append(m)
    return in_maps


def kernel(timestamps, features, amplitudes, neuron_ids,
           We1, be1, We2, be2, g_ev, b_ev,
           Wt1, bt1, Wt2, bt2, Wc, bc, g_sr, b_sr,
           Wq, bq, Wk, bk, Wv, bv, **_unused):
    del neuron_ids  # unused by the reference model
    nc = _get_nc()
    in_maps = build_in_maps(
        timestamps, features, amplitudes,
        We1, be1, We2, be2, g_ev, b_ev,
        Wt1, bt1, Wt2, bt2, Wc, bc, g_sr, b_sr,
        Wq, bq, Wk, bk, Wv, bv)
    res = run_bass_kernel_spmd(nc, in_maps, core_ids=list(range(N_CORES)))
    return assemble_outputs(res.results)


def assemble_outputs(outs):
    K = np.concatenate([outs[c]["K_sh"] for c in range(N_CORES)], axis=1)
    Q = np.concatenate([outs[c]["q_out"] for c in range(N_CORES)],
                       axis=0).reshape(1, S, H)
    V = np.concatenate([outs[c]["v_out"] for c in range(N_CORES)],
                       axis=0).reshape(1, S, H)
    return Q, K, V


# revision 20
# speedup vs baseline: 1.3292x; 1.0009x over previous
"""Trainium2 Bass kernel for nn_NeurosynapticEventEncoder.

Reference model:
    sort events by timestamp -> event MLP + LN -> temporal MLP ->
    concat/amp-gate -> proj 2H->H + LN -> + sinusoidal pos enc -> ctx
    Q = ctx@Wq+bq, Kb = ctx@Wk+bk, V = ctx@Wv+bv,
    K[i,j,h] = Kb[j,h] + 0.1*conn[i,j]  (conn = 1/(1+|i-j|))

Sharding: K [S,S,H] (536 MB fp32) is sharded over its SECOND axis (j)
across 8 cores.  Everything upstream of K is row-wise in s (LayerNorms
are per-row, matmuls are row-independent), so each core computes only
its 64-row slice of the whole chain — ctx/Kb/Q/V shards — with no
collectives and no redundant compute.  Host concatenates Q/V (axis 0)
and K (axis 1).

Device layout: the MLP chain keeps activations transposed (X^T, [D, s])
so matmuls use weight matrices as-stored for the stationary operand
(out = lhsT.T @ rhs).  The 2H->H projection flips orientation
(lhsT = comb^T) to produce `post` as [s, H], which makes LayerNorm 2 a
native free-dim bn_stats and lets the amplitude gate ride the PSUM->SBUF
copy as a per-partition scale.  ctx is then transposed once via the PE.
K expansion: K_flat[(i,jl), h] = kb[jl, h] + 0.1*conn[i, j0+jl]; each
[128, 512] slab covers two i values (kb duplicated into both partition
halves), written out as 4 MB contiguous DMAs.

Host-side prep (index/constant work only): argsort by timestamps and
gather, transposed feature shard, sinusoidal table, conn table, and the
exact weight folds Wc_eff = [Wc[:512]; Wc[512:768]+Wc[768:]] (the two
tiled t blocks see identical activations) and te' = te + b_sr.
"""

import math
from contextlib import ExitStack

import numpy as np

import concourse.bass as bass
import concourse.tile as tile
from concourse import bacc, mybir
from concourse.bass_utils import run_bass_kernel_spmd
from concourse.masks import make_identity

S = 512
D_IN = 256
H = 512
N_CORES = 8
J_SH = S // N_CORES       # 64 columns of K / rows of ctx per core
NBLK = S * J_SH // 128    # 256 slabs of [128, 512] per core
BLK_PER_TILE = 32         # 32 slabs -> 8.4 MB per K-write DMA
F32 = mybir.dt.float32
AF = mybir.ActivationFunctionType
ALU = mybir.AluOpType
EPS = 1e-5


# ----------------------------------------------------------------------------
# device program
# ----------------------------------------------------------------------------

def _layer_norm_part(nc, pool, ps_red, ps_bc, x_ap, g_sb, b_sb, out_ap,
                     ones_col, ones_row, eps_sb):
    """LN over the partition axis (nkt x [128, n] tiles), transposed layout."""
    nkt, n = x_ap.shape[1], x_ap.shape[2]
    cs_ps = ps_red.tile([1, n], F32, name="cs_ps")
    for kt in range(nkt):
        nc.tensor.matmul(cs_ps, ones_col, x_ap[:, kt, :],
                         start=(kt == 0), stop=(kt == nkt - 1))
    csq_ps = ps_red.tile([1, n], F32, name="csq_ps")
    for kt in range(nkt):
        sq = pool.tile([128, n], F32, name="sq_scratch", bufs=2)
        nc.vector.tensor_mul(sq, x_ap[:, kt, :], x_ap[:, kt, :])
        nc.tensor.matmul(csq_ps, ones_col, sq,
                         start=(kt == 0), stop=(kt == nkt - 1))
    mean_r = pool.tile([1, n], F32, name="mean_r")
    nc.scalar.activation(mean_r, cs_ps, AF.Copy, bias=0.0, scale=1.0 / H)
    msq_r = pool.tile([1, n], F32, name="msq_r")
    nc.scalar.activation(msq_r, csq_ps, AF.Copy, bias=0.0, scale=1.0 / H)
    m2_r = pool.tile([1, n], F32, name="m2_r")
    nc.vector.tensor_mul(m2_r, mean_r, mean_r)
    var_r = pool.tile([1, n], F32, name="var_r")
    nc.vector.tensor_sub(var_r, msq_r, m2_r)
    std_r = pool.tile([1, n], F32, name="std_r")
    nc.scalar.activation(std_r, var_r, AF.Sqrt, bias=eps_sb[0:1, 0:1], scale=1.0)
    rstd_r = pool.tile([1, n], F32, name="rstd_r")
    nc.vector.reciprocal(rstd_r, std_r)
    mrs_r = pool.tile([1, n], F32, name="mrs_r")
    nc.vector.tensor_mul(mrs_r, mean_r, rstd_r)
    a_ps = ps_bc.tile([128, n], F32, name="a_ps", tag="bc_ps")
    nc.tensor.matmul(a_ps, ones_row, rstd_r, start=True, stop=True)
    m_ps = ps_bc.tile([128, n], F32, name="m_ps", tag="bc_ps")
    nc.tensor.matmul(m_ps, ones_row, mrs_r, start=True, stop=True)
    a_sb = pool.tile([128, n], F32, name="a_sb")
    nc.scalar.copy(a_sb, a_ps)
    m_sb = pool.tile([128, n], F32, name="m_sb")
    nc.scalar.copy(m_sb, m_ps)
    for kt in range(nkt):
        t1 = pool.tile([128, n], F32, name="ln_t1", bufs=2)
        nc.vector.tensor_mul(t1, x_ap[:, kt, :], a_sb)
        nc.vector.tensor_sub(t1, t1, m_sb)
        nc.vector.tensor_scalar(
            out=out_ap[:, kt, :], in0=t1,
            scalar1=g_sb[:, kt:kt + 1], scalar2=b_sb[:, kt:kt + 1],
            op0=ALU.mult, op1=ALU.add)


def _build_nc():
    nc = bacc.Bacc()
    n = J_SH  # per-core s extent

    def inp(name, shape):
        return nc.declare_dram_parameter(name, list(shape), F32, isOutput=False)

    # per-core shards
    featsT_d = inp("featsT_sh", (D_IN, n))
    ts_d = inp("ts_sh", (n,))
    amps_d = inp("amps_sh", (n,))
    te_d = inp("te_sh", (n, H))          # te[s_shard] + b_sr
    C_d = inp("conn2_sh", (128, NBLK))   # 0.1*conn arranged per slab
    # replicated weights
    We1_d = inp("We1", (D_IN, H // 2))
    We2_d = inp("We2", (H // 2, H))
    Wt2_d = inp("Wt2", (H // 4, H // 2))
    Wc_d = inp("Wc_eff", (H + H // 2, H))
    Wq_d = inp("Wq", (H, H))
    Wk_d = inp("Wk", (H, H))
    Wv_d = inp("Wv", (H, H))
    # packed small vectors: per-partition [128, 17] and rows [1, 2688]
    pp_d = inp("pp_pack", (128, 17))
    row_d = inp("row_pack", (1, H // 4 + 5 * H))

    Ksh_d = nc.declare_dram_parameter("K_sh", [S, J_SH, H], F32, isOutput=True)
    q_d = nc.declare_dram_parameter("q_out", [n, H], F32, isOutput=True)
    v_d = nc.declare_dram_parameter("v_out", [n, H], F32, isOutput=True)

    with tile.TileContext(nc) as tc:
        with ExitStack() as ctx:
            persist = ctx.enter_context(tc.tile_pool(name="persist", bufs=1))
            ps_mm = ctx.enter_context(
                tc.tile_pool(name="ps_mm", bufs=2, space=bass.MemorySpace.PSUM))
            ps_red = ctx.enter_context(
                tc.tile_pool(name="ps_red", bufs=1, space=bass.MemorySpace.PSUM))
            ps_bc = ctx.enter_context(
                tc.tile_pool(name="ps_bc", bufs=2, space=bass.MemorySpace.PSUM))

            ones_col = persist.tile([128, 1], F32)
            nc.vector.memset(ones_col, 1.0)
            ones_row = persist.tile([1, 128], F32)
            nc.vector.memset(ones_row, 1.0)
            eps_sb = persist.tile([128, 1], F32)
            nc.vector.memset(eps_sb, EPS)
            ident = persist.tile([128, 128], F32)
            make_identity(nc, ident)
            kb2_sb = persist.tile([128, H], F32)    # kb duplicated in both halves
            ctxT_sb = persist.tile([128, 4, n], F32)
            C_sb = persist.tile([128, NBLK], F32)

            with tc.tile_pool(name="p1", bufs=1) as p1:
                def load(name, shape, dram_ap, engine=None):
                    t = p1.tile(shape, F32, name=name)
                    (engine or nc.sync).dma_start(out=t, in_=dram_ap)
                    return t

                # ordered: first-needed first; small vectors packed
                feats_sb = load("feats_sb", [128, 2, n],
                                featsT_d[:].rearrange("(kt p) s -> p kt s", p=128))
                We1_sb = load("We1_sb", [128, 2, H // 2],
                              We1_d[:].rearrange("(kt p) m -> p kt m", p=128))
                pp_sb = load("pp_sb", [128, 17], pp_d[:], nc.gpsimd)
                be1_sb = pp_sb[:, 0:2]
                bt1_sb = pp_sb[:, 2:3]
                bt2_sb = pp_sb[:, 3:5]
                be2_sb = pp_sb[:, 5:9]
                gev_sb = pp_sb[:, 9:13]
                bev_sb = pp_sb[:, 13:17]
                row_sb = load("row_sb", [1, H // 4 + 5 * H], row_d[:])
                Wt1_sb = row_sb[:, 0:128]
                bc_sb = row_sb[:, 128:640]
                gsr_sb = row_sb[:, 640:1152]
                bq_sb = row_sb[:, 1152:1664]
                bk_sb = row_sb[:, 1664:2176]
                bv_sb = row_sb[:, 2176:2688]
                ts_sb = load("ts_sb", [1, n], ts_d[:].unsqueeze(0), nc.gpsimd)
                amps_sb = load("amps_sb", [n, 1], amps_d[:].unsqueeze(-1),
                               nc.gpsimd)
                We2_sb = load("We2_sb", [128, 2, H],
                              We2_d[:].rearrange("(kt p) m -> p kt m", p=128))
                Wt2_sb = load("Wt2_sb", [128, H // 2], Wt2_d[:])
                Wc_sb = load("Wc_sb", [128, 6, H],
                             Wc_d[:].rearrange("(kt p) m -> p kt m", p=128))
                te_sb = load("te_sb", [n, H], te_d[:])
                Wk_sb = load("Wk_sb", [128, 4, H],
                             Wk_d[:].rearrange("(kt p) m -> p kt m", p=128))
                Wq_sb = load("Wq_sb", [128, 4, H],
                             Wq_d[:].rearrange("(kt p) m -> p kt m", p=128))
                Wv_sb = load("Wv_sb", [128, 4, H],
                             Wv_d[:].rearrange("(kt p) m -> p kt m", p=128))
                nc.sync.dma_start(out=C_sb, in_=C_d[:])

                # ---- event MLP (transposed: [D, s]) ----
                h1_sb = p1.tile([128, 2, n], F32)
                for mt in range(2):
                    mm_ps = ps_mm.tile([128, n], F32, name="mm_sm")
                    for kt in range(2):
                        nc.tensor.matmul(
                            mm_ps, We1_sb[:, kt, bass.ts(mt, 128)],
                            feats_sb[:, kt, :], start=(kt == 0), stop=(kt == 1))
                    nc.scalar.activation(h1_sb[:, mt, :], mm_ps, AF.Relu,
                                         bias=be1_sb[:, mt:mt + 1], scale=1.0)
                h2_sb = p1.tile([128, 4, n], F32)
                for mt in range(4):
                    mm_ps = ps_mm.tile([128, n], F32, name="mm_sm")
                    for kt in range(2):
                        nc.tensor.matmul(
                            mm_ps, We2_sb[:, kt, bass.ts(mt, 128)],
                            h1_sb[:, kt, :], start=(kt == 0), stop=(kt == 1))
                    nc.scalar.activation(h2_sb[:, mt, :], mm_ps, AF.Identity,
                                         bias=be2_sb[:, mt:mt + 1], scale=1.0)

                # ---- temporal MLP ----
                t1_ps = ps_mm.tile([128, n], F32, name="mm_sm")
                nc.tensor.matmul(t1_ps, Wt1_sb, ts_sb, start=True, stop=True)
                t1_sb = p1.tile([128, n], F32)
                nc.scalar.activation(t1_sb, t1_ps, AF.Relu,
                                     bias=bt1_sb[:, 0:1], scale=1.0)
                t2_sb = p1.tile([128, 2, n], F32)
                for mt in range(2):
                    mm_ps = ps_mm.tile([128, n], F32, name="mm_sm")
                    nc.tensor.matmul(mm_ps, Wt2_sb[:, bass.ts(mt, 128)], t1_sb,
                                     start=True, stop=True)
                    nc.scalar.activation(t2_sb[:, mt, :], mm_ps, AF.Identity,
                                         bias=bt2_sb[:, mt:mt + 1], scale=1.0)

                # ---- LN1 (over H = partition axis) ----
                c1_sb = p1.tile([128, 4, n], F32)
                _layer_norm_part(nc, p1, ps_red, ps_bc, h2_sb, gev_sb, bev_sb,
                                 c1_sb, ones_col, ones_row, eps_sb)

                # ---- comb @ Wc_eff + bc, then amp-gate on the copy ----
                comb_kt = [c1_sb[:, 0, :], c1_sb[:, 1, :], c1_sb[:, 2, :],
                           c1_sb[:, 3, :], t2_sb[:, 0, :], t2_sb[:, 1, :]]
                post_ps = ps_mm.tile([n, H], F32, name="mm_big")
                for kt in range(6):
                    nc.tensor.matmul(post_ps, comb_kt[kt], Wc_sb[:, kt, :],
                                     start=(kt == 0), stop=False)
                nc.tensor.matmul(post_ps, ones_row[:, 0:n], bc_sb,
                                 start=False, stop=True)
                post_sb = p1.tile([n, H], F32)
                nc.scalar.activation(post_sb, post_ps, AF.Copy,
                                     bias=0.0, scale=amps_sb)

                # ---- LN2 (over H = free axis, bn_stats) + g_sr + (te + b_sr) --
                stats = p1.tile([n, nc.vector.BN_STATS_DIM], F32)
                nc.vector.bn_stats(out=stats, in_=post_sb)
                mv = p1.tile([n, nc.vector.BN_AGGR_DIM], F32)
                nc.vector.bn_aggr(out=mv, in_=stats)
                std_c = p1.tile([n, 1], F32)
                nc.scalar.activation(std_c, mv[:, 1:2], AF.Sqrt,
                                     bias=eps_sb[0:n, 0:1], scale=1.0)
                rstd_c = p1.tile([n, 1], F32)
                nc.vector.reciprocal(rstd_c, std_c)
                xn_sb = p1.tile([n, H], F32)
                nc.vector.tensor_scalar(out=xn_sb, in0=post_sb,
                                        scalar1=mv[:, 0:1], scalar2=rstd_c,
                                        op0=ALU.subtract, op1=ALU.mult)
                g_ps = ps_mm.tile([n, H], F32, name="g_ps", tag="mm_big")
                nc.tensor.matmul(g_ps, ones_row[:, 0:n], gsr_sb,
                                 start=True, stop=True)
                g_bc = p1.tile([n, H], F32)
                nc.scalar.copy(g_bc, g_ps)
                ctx_sb = p1.tile([n, H], F32)
                nc.vector.tensor_mul(ctx_sb, xn_sb, g_bc)
                nc.vector.tensor_add(ctx_sb, ctx_sb, te_sb)

                # ---- transpose ctx -> ctx^T [H, s] ----
                for ht in range(4):
                    tp_ps = ps_bc.tile([128, n], F32, name="tp_ps", tag="bc_ps")
                    nc.tensor.transpose(tp_ps, ctx_sb[:, bass.ts(ht, 128)],
                                        ident[0:n, 0:n])
                    nc.scalar.copy(ctxT_sb[:, ht, :], tp_ps)

                # ---- projections: out[s_local, h] = ctx@W + b ----
                def project(W_sb, b_row, out_sb):
                    mm_ps = ps_mm.tile([n, H], F32, name="mm_big")
                    for kt in range(4):
                        nc.tensor.matmul(mm_ps, ctxT_sb[:, kt, :], W_sb[:, kt, :],
                                         start=(kt == 0), stop=False)
                    nc.tensor.matmul(mm_ps, ones_row[:, 0:n], b_row,
                                     start=False, stop=True)
                    nc.scalar.copy(out_sb, mm_ps)

                project(Wk_sb, bk_sb, kb2_sb[0:n, :])
                # duplicate kb into partitions 64..127 (slabs cover 2 i's)
                nc.gpsimd.dma_start(out=kb2_sb[n:2 * n, :], in_=kb2_sb[0:n, :])

                q_sb = p1.tile([n, H], F32)
                project(Wq_sb, bq_sb, q_sb)
                nc.sync.dma_start(out=q_d[:], in_=q_sb)
                v_sb = p1.tile([n, H], F32)
                project(Wv_sb, bv_sb, v_sb)
                nc.sync.dma_start(out=v_d[:], in_=v_sb)

            # ---- K expansion ----
            # K_sh[i, jl, h]; flat row i*64+jl; partition p=(i%2)*64+jl.
            kw = ctx.enter_context(tc.tile_pool(name="kw", bufs=2))
            Kv = Ksh_d[:].rearrange("(blk a) j h -> (a j) blk h", a=2)
            nslab = 0
            for b0 in range(0, NBLK, BLK_PER_TILE):
                kT = kw.tile([128, BLK_PER_TILE, H], F32, name="kT")
                for m in range(BLK_PER_TILE):
                    blk = b0 + m
                    slab = kT[:, m, :]
                    if nslab % 5 < 3:
                        nc.vector.tensor_scalar_add(slab, kb2_sb,
                                                    C_sb[:, blk:blk + 1])
                    else:
                        nc.scalar.add(slab, kb2_sb, C_sb[:, blk:blk + 1])
                    nslab += 1
                nc.sync.dma_start(out=Kv[:, b0:b0 + BLK_PER_TILE, :], in_=kT)

    nc.finalize()  # Bacc: wait-splitting + register allocation passes
    return nc


_NC = None


def _get_nc():
    global _NC
    if _NC is None:
        _NC = _build_nc()
    return _NC


# ----------------------------------------------------------------------------
# host side
# ----------------------------------------------------------------------------

def _host_constants():
    pos = np.arange(S, dtype=np.float32)[:, None]
    div = np.exp(np.arange(0, H, 2, dtype=np.float32)
                 * np.float32(-math.log(10000.0) / H))
    ang = pos * div[None, :]
    te = np.stack([np.sin(ang), np.cos(ang)], axis=-1).reshape(S, H)
    te = te.astype(np.float32)
    idx = np.arange(S)
    conn = (1.0 / (1.0 + np.abs(idx[:, None] - idx[None, :]).astype(np.float32)))
    C = (0.1 * conn).astype(np.float32)
    return te, C


def _f32(x):
    return np.ascontiguousarray(np.asarray(x, dtype=np.float32))


def build_in_maps(timestamps, features, amplitudes,
                  We1, be1, We2, be2, g_ev, b_ev,
                  Wt1, bt1, Wt2, bt2, Wc, bc, g_sr, b_sr,
                  Wq, bq, Wk, bk, Wv, bv):
    timestamps = _f32(timestamps)
    order = np.argsort(timestamps, kind="stable")
    ts_s = timestamps[order]
    featsT = np.ascontiguousarray(_f32(features)[order].T)  # [D_IN, S]
    amps_s = _f32(amplitudes)[order]
    te, C = _host_constants()
    b_sr = _f32(b_sr)
    Wc = _f32(Wc)
    # exact folds: the two tiled t blocks see identical activations
    Wc_eff = np.ascontiguousarray(
        np.concatenate([Wc[:H], Wc[H:H + H // 2] + Wc[H + H // 2:]], axis=0))
    te_b = te + b_sr[None, :]  # fold LN2 shift into the te add

    # per-partition pack [128, 17]: be1(2) bt1(1) bt2(2) be2(4) g_ev(4) b_ev(4)
    def cols(v, t):  # [t*128] -> [128, t]
        return _f32(v).reshape(t, 128).T
    pp = np.concatenate(
        [cols(be1, 2), cols(bt1, 1), cols(bt2, 2), cols(be2, 4),
         cols(g_ev, 4), cols(b_ev, 4)], axis=1)
    row = np.concatenate(
        [_f32(Wt1).reshape(-1), _f32(bc), _f32(g_sr),
         _f32(bq), _f32(bk), _f32(bv)])[None, :]
    common = {
        "We1": _f32(We1), "We2": _f32(We2), "Wt2": _f32(Wt2),
        "Wc_eff": Wc_eff, "Wq": _f32(Wq), "Wk": _f32(Wk), "Wv": _f32(Wv),
        "pp_pack": np.ascontiguousarray(pp),
        "row_pack": np.ascontiguousarray(row),
    }
    in_maps = []
    for c in range(N_CORES):
        sh = slice(c * J_SH, (c + 1) * J_SH)
        # conn2[p=(i%2)*64+jl, blk=i//2] = 0.1*conn[i, c*64+jl]
        base = C[:, sh]                       # [i=512, jl=64]
        conn2 = np.ascontiguousarray(
            base.reshape(NBLK, 2, J_SH).transpose(1, 2, 0).reshape(128, NBLK))
        m = dict(common)
        m["featsT_sh"] = np.ascontiguousarray(featsT[:, sh])
        m["ts_sh"] = np.ascontiguousarray(ts_s[sh])
        m["amps_sh"] = np.ascontiguousarray(amps_s[sh])
        m["te_sh"] = np.ascontiguousarray(te_b[sh])
        m["conn2_sh"] = conn2
        in_maps.append(m)
    return in_maps


def kernel(timestamps, features, amplitudes, neuron_ids,
           We1, be1, We2, be2, g_ev, b_ev,
           Wt1, bt1, Wt2, bt2, Wc, bc, g_sr, b_sr,
           Wq, bq, Wk, bk, Wv, bv, **_unused):
    del neuron_ids  # unused by the reference model
    nc = _get_nc()
    in_maps = build_in_maps(
        timestamps, features, amplitudes,
        We1, be1, We2, be2, g_ev, b_ev,
        Wt1, bt1, Wt2, bt2, Wc, bc, g_sr, b_sr,
        Wq, bq, Wk, bk, Wv, bv)
    res = run_bass_kernel_spmd(nc, in_maps, core_ids=list(range(N_CORES)))
    return assemble_outputs(res.results)


def assemble_outputs(outs):
    K = np.concatenate([outs[c]["K_sh"] for c in range(N_CORES)], axis=1)
    Q = np.concatenate([outs[c]["q_out"] for c in range(N_CORES)],
                       axis=0).reshape(1, S, H)
    V = np.concatenate([outs[c]["v_out"] for c in range(N_CORES)],
                       axis=0).reshape(1, S, H)
    return Q, K, V


# revision 24
# speedup vs baseline: 1.3321x; 1.0022x over previous
"""Trainium2 Bass kernel for nn_NeurosynapticEventEncoder.

Reference model:
    sort events by timestamp -> event MLP + LN -> temporal MLP ->
    concat/amp-gate -> proj 2H->H + LN -> + sinusoidal pos enc -> ctx
    Q = ctx@Wq+bq, Kb = ctx@Wk+bk, V = ctx@Wv+bv,
    K[i,j,h] = Kb[j,h] + 0.1*conn[i,j]  (conn = 1/(1+|i-j|))

Sharding: K [S,S,H] (536 MB fp32) is sharded over its SECOND axis (j)
across 8 cores.  Everything upstream of K is row-wise in s (LayerNorms
are per-row, matmuls row-independent), so each core computes only its
64-row slice of the whole chain — ctx/Kb/Q/V shards — no collectives,
no redundant compute.  Host concatenates Q/V (axis 0) and K (axis 1).

Device pipeline (per core, s-shard n=64):
  MLP1 transposed ([D, s], weights-as-stored lhsT) -> h1T
  MLP2 flipped (lhsT=h1T) -> h2 [s, H]  -> LN1 via bn_stats -> c1n
  PE-transpose c1n -> c1T; temporal MLP transposed -> t2T
  comb^T @ Wc' (+bc' bias row) -> amp-gate on the PSUM->SBUF copy
  LN2 via bn_stats -> xn [s, H] -> PE-transpose -> xnT
  Kb/Q/V = xn @ W'_p + teW_p  (teW accumulated into PSUM by an
  identity-weight matmul)
  K expansion: K_flat[(i,jl),h] = kb[jl,h] + 0.1conn[i,j0+jl] as
  [128,512] per-partition-scalar adds (DVE/ACT), 8 MB coalesced DMAs.

Exact host-side algebra folds (weight preprocessing only):
  Wc' = [g_ev*Wc[:H]; Wc[H:H+H/2]+Wc[H+H/2:]]   (t is tiled twice)
  bc' = bc + b_ev@Wc[:H]
  W'_p = g_sr*W_p;  teW_p = (te[sh]+b_sr)@W'_p + b_p  (p in {q,k,v})
"""

import math
from contextlib import ExitStack

import numpy as np

import concourse.bass as bass
import concourse.tile as tile
from concourse import bacc, mybir
from concourse.bass_utils import run_bass_kernel_spmd
from concourse.masks import make_identity

S = 512
D_IN = 256
H = 512
N_CORES = 8
J_SH = S // N_CORES       # 64 columns of K / rows of ctx per core
NBLK = S * J_SH // 128    # 256 slabs of [128, 512] per core
F32 = mybir.dt.float32
AF = mybir.ActivationFunctionType
ALU = mybir.AluOpType
EPS = 1e-5
# K-write tile sizes in slabs: small first tiles to start DMA early
KW_TILES = [8, 8, 8, 8] + [32] * 7


def _build_nc():
    nc = bacc.Bacc()
    n = J_SH

    def inp(name, shape):
        return nc.declare_dram_parameter(name, list(shape), F32, isOutput=False)

    # per-core shards
    featsT_d = inp("featsT_sh", (D_IN, n))
    ts_d = inp("ts_sh", (n,))
    amps_d = inp("amps_sh", (n,))
    teWq_d = inp("teWq_sh", (n, H))
    teWk_d = inp("teWk_sh", (n, H))
    teWv_d = inp("teWv_sh", (n, H))
    C_d = inp("conn2_sh", (128, NBLK))
    # replicated (pre-folded) weights
    We1_d = inp("We1", (D_IN, H // 2))
    We2_d = inp("We2", (H // 2, H))
    Wt2_d = inp("Wt2", (H // 4, H // 2))
    Wc_d = inp("Wc_eff", (H + H // 2, H))
    Wq_d = inp("Wq_g", (H, H))
    Wk_d = inp("Wk_g", (H, H))
    Wv_d = inp("Wv_g", (H, H))
    pp_d = inp("pp_pack", (128, 5))      # be1(2) bt1(1) bt2(2)
    row_d = inp("row_pack", (1, H // 4 + 2 * H))  # Wt1 | bc' | be2

    Ksh_d = nc.declare_dram_parameter("K_sh", [S, J_SH, H], F32, isOutput=True)
    q_d = nc.declare_dram_parameter("q_out", [n, H], F32, isOutput=True)
    v_d = nc.declare_dram_parameter("v_out", [n, H], F32, isOutput=True)

    with tile.TileContext(nc) as tc:
        with ExitStack() as ctx:
            persist = ctx.enter_context(tc.tile_pool(name="persist", bufs=1))
            ps_mm = ctx.enter_context(
                tc.tile_pool(name="ps_mm", bufs=2, space=bass.MemorySpace.PSUM))
            ps_sm = ctx.enter_context(
                tc.tile_pool(name="ps_sm", bufs=2, space=bass.MemorySpace.PSUM))

            ones_row = persist.tile([1, 128], F32)
            nc.vector.memset(ones_row, 1.0)
            eps_sb = persist.tile([128, 1], F32)
            nc.vector.memset(eps_sb, EPS)
            ident = persist.tile([128, 128], F32)
            make_identity(nc, ident)
            kb2_sb = persist.tile([128, H], F32)
            C_sb = persist.tile([128, NBLK], F32)

            with tc.tile_pool(name="p1", bufs=1) as p1:
                def load(name, shape, dram_ap, engine=None):
                    t = p1.tile(shape, F32, name=name)
                    (engine or nc.sync).dma_start(out=t, in_=dram_ap)
                    return t

                feats_sb = load("feats_sb", [128, 2, n],
                                featsT_d[:].rearrange("(kt p) s -> p kt s", p=128))
                We1_sb = load("We1_sb", [128, 2, H // 2],
                              We1_d[:].rearrange("(kt p) m -> p kt m", p=128))
                pp_sb = load("pp_sb", [128, 5], pp_d[:], nc.gpsimd)
                be1_sb = pp_sb[:, 0:2]
                bt1_sb = pp_sb[:, 2:3]
                bt2_sb = pp_sb[:, 3:5]
                row_sb = load("row_sb", [1, H // 4 + 2 * H], row_d[:])
                Wt1_sb = row_sb[:, 0:128]
                bc_sb = row_sb[:, 128:640]
                be2_sb = row_sb[:, 640:1152]
                ts_sb = load("ts_sb", [1, n], ts_d[:].unsqueeze(0), nc.gpsimd)
                amps_sb = load("amps_sb", [n, 1], amps_d[:].unsqueeze(-1),
                               nc.gpsimd)
                amps_row = load("amps_row", [1, n], amps_d[:].unsqueeze(0),
                                nc.gpsimd)
                We2_sb = load("We2_sb", [128, 2, H],
                              We2_d[:].rearrange("(kt p) m -> p kt m", p=128))
                Wt2_sb = load("Wt2_sb", [128, H // 2], Wt2_d[:])
                Wc_sb = load("Wc_sb", [128, 6, H],
                             Wc_d[:].rearrange("(kt p) m -> p kt m", p=128))
                Wk_sb = load("Wk_sb", [128, 4, H],
                             Wk_d[:].rearrange("(kt p) m -> p kt m", p=128))
                teWk_sb = load("teWk_sb", [n, H], teWk_d[:])
                Wq_sb = load("Wq_sb", [128, 4, H],
                             Wq_d[:].rearrange("(kt p) m -> p kt m", p=128))
                teWq_sb = load("teWq_sb", [n, H], teWq_d[:])
                Wv_sb = load("Wv_sb", [128, 4, H],
                             Wv_d[:].rearrange("(kt p) m -> p kt m", p=128))
                teWv_sb = load("teWv_sb", [n, H], teWv_d[:])
                nc.sync.dma_start(out=C_sb, in_=C_d[:])

                # ---- event MLP layer 1 (transposed): h1T [H/2, s] ----
                h1_sb = p1.tile([128, 2, n], F32)
                for mt in range(2):
                    mm = ps_sm.tile([128, n], F32, name="mm_sm")
                    for kt in range(2):
                        nc.tensor.matmul(
                            mm, We1_sb[:, kt, bass.ts(mt, 128)],
                            feats_sb[:, kt, :], start=(kt == 0), stop=(kt == 1))
                    nc.scalar.activation(h1_sb[:, mt, :], mm, AF.Relu,
                                         bias=be1_sb[:, mt:mt + 1], scale=1.0)

                # ---- temporal MLP (transposed): t2T [H/2, s] ----
                t1_ps = ps_sm.tile([128, n], F32, name="mm_sm")
                nc.tensor.matmul(t1_ps, Wt1_sb, ts_sb, start=True, stop=True)
                t1_sb = p1.tile([128, n], F32)
                nc.scalar.activation(t1_sb, t1_ps, AF.Relu,
                                     bias=bt1_sb[:, 0:1], scale=1.0)
                t2_sb = p1.tile([128, 2, n], F32)
                for mt in range(2):
                    mm = ps_sm.tile([128, n], F32, name="mm_sm")
                    nc.tensor.matmul(mm, Wt2_sb[:, bass.ts(mt, 128)], t1_sb,
                                     start=True, stop=True)
                    nc.scalar.activation(t2_sb[:, mt, :], mm, AF.Identity,
                                         bias=bt2_sb[:, mt:mt + 1], scale=1.0)

                # ---- event MLP layer 2 (flipped): h2 [s, H] ----
                h2_ps = ps_mm.tile([n, H], F32, name="mm_big")
                for kt in range(2):
                    nc.tensor.matmul(h2_ps, h1_sb[:, kt, :], We2_sb[:, kt, :],
                                     start=(kt == 0), stop=False)
                nc.tensor.matmul(h2_ps, ones_row[:, 0:n], be2_sb,
                                 start=False, stop=True)
                h2_sb = p1.tile([n, H], F32)
                nc.scalar.copy(h2_sb, h2_ps)

                # ---- LN1 (free axis, bn_stats); g/b folded into Wc'/bc' ----
                def ln_normalize(x_sb, out_sb, tagn):
                    stats = p1.tile([n, nc.vector.BN_STATS_DIM], F32,
                                    name=f"st{tagn}")
                    nc.vector.bn_stats(out=stats, in_=x_sb)
                    mv = p1.tile([n, nc.vector.BN_AGGR_DIM], F32,
                                 name=f"mv{tagn}")
                    nc.vector.bn_aggr(out=mv, in_=stats)
                    std_c = p1.tile([n, 1], F32, name=f"sd{tagn}")
                    nc.scalar.activation(std_c, mv[:, 1:2], AF.Sqrt,
                                         bias=eps_sb[0:n, 0:1], scale=1.0)
                    rstd_c = p1.tile([n, 1], F32, name=f"rs{tagn}")
                    nc.vector.reciprocal(rstd_c, std_c)
                    nc.vector.tensor_scalar(out=out_sb, in0=x_sb,
                                            scalar1=mv[:, 0:1], scalar2=rstd_c,
                                            op0=ALU.subtract, op1=ALU.mult)

                c1n_sb = p1.tile([n, H], F32)
                ln_normalize(h2_sb, c1n_sb, "1")

                # transpose c1n -> c1T [H, s]
                c1T_sb = p1.tile([128, 4, n], F32)
                for ht in range(4):
                    tp = ps_sm.tile([128, n], F32, name="mm_sm")
                    nc.tensor.transpose(tp, c1n_sb[:, bass.ts(ht, 128)],
                                        ident[0:n, 0:n])
                    nc.scalar.copy(c1T_sb[:, ht, :], tp)

                # ---- comb^T @ Wc'; +bc'/amps via inv-amps outer product so
                # the amp-gate on the copy restores an UNgated bc' ----
                ia_sb = p1.tile([1, n], F32)
                nc.vector.reciprocal(ia_sb, amps_row)
                comb_kt = [c1T_sb[:, 0, :], c1T_sb[:, 1, :], c1T_sb[:, 2, :],
                           c1T_sb[:, 3, :], t2_sb[:, 0, :], t2_sb[:, 1, :]]
                post_ps = ps_mm.tile([n, H], F32, name="mm_big")
                for kt in range(6):
                    nc.tensor.matmul(post_ps, comb_kt[kt], Wc_sb[:, kt, :],
                                     start=(kt == 0), stop=False)
                nc.tensor.matmul(post_ps, ia_sb, bc_sb,
                                 start=False, stop=True)
                post_sb = p1.tile([n, H], F32)
                nc.scalar.activation(post_sb, post_ps, AF.Copy,
                                     bias=0.0, scale=amps_sb)

                # ---- LN2 -> xn [s, H]; transpose -> xnT ----
                xn_sb = p1.tile([n, H], F32)
                ln_normalize(post_sb, xn_sb, "2")
                xnT_sb = p1.tile([128, 4, n], F32)
                for ht in range(4):
                    tp = ps_sm.tile([128, n], F32, name="mm_sm")
                    nc.tensor.transpose(tp, xn_sb[:, bass.ts(ht, 128)],
                                        ident[0:n, 0:n])
                    nc.scalar.copy(xnT_sb[:, ht, :], tp)

                # ---- projections: out = xn @ W'_p + teW_p ----
                def project(W_sb, teW_sb, out_sb):
                    mm = ps_mm.tile([n, H], F32, name="mm_big")
                    for kt in range(4):
                        nc.tensor.matmul(mm, xnT_sb[:, kt, :], W_sb[:, kt, :],
                                         start=(kt == 0), stop=False)
                    nc.tensor.matmul(mm, ident[0:n, 0:n], teW_sb,
                                     start=False, stop=True)
                    nc.scalar.copy(out_sb, mm)

                project(Wk_sb, teWk_sb, kb2_sb[0:n, :])
                nc.gpsimd.dma_start(out=kb2_sb[n:2 * n, :], in_=kb2_sb[0:n, :])

                q_sb = p1.tile([n, H], F32)
                project(Wq_sb, teWq_sb, q_sb)
                nc.sync.dma_start(out=q_d[:], in_=q_sb)
                v_sb = p1.tile([n, H], F32)
                project(Wv_sb, teWv_sb, v_sb)
                nc.sync.dma_start(out=v_d[:], in_=v_sb)

            # ---- K expansion ----
            # K_sh[i, jl, h]; flat row i*64+jl; partition p=(i%2)*64+jl.
            kw = ctx.enter_context(tc.tile_pool(name="kw", bufs=2))
            Kv = Ksh_d[:].rearrange("(blk a) j h -> (a j) blk h", a=2)
            nslab = 0
            b0 = 0
            for tsize in KW_TILES:
                kT = kw.tile([128, tsize, H], F32,
                             name=f"kT{tsize}", tag=f"kT{tsize}")
                for m in range(tsize):
                    blk = b0 + m
                    slab = kT[:, m, :]
                    if nslab % 5 < 3:
                        nc.vector.tensor_scalar_add(slab, kb2_sb,
                                                    C_sb[:, blk:blk + 1])
                    else:
                        nc.scalar.add(slab, kb2_sb, C_sb[:, blk:blk + 1])
                    nslab += 1
                nc.sync.dma_start(out=Kv[:, b0:b0 + tsize, :], in_=kT)
                b0 += tsize
            assert b0 == NBLK

    nc.finalize()  # Bacc: wait-splitting + register allocation passes
    return nc


_NC = None


def _get_nc():
    global _NC
    if _NC is None:
        _NC = _build_nc()
    return _NC


# ----------------------------------------------------------------------------
# host side
# ----------------------------------------------------------------------------

def _host_constants():
    pos = np.arange(S, dtype=np.float32)[:, None]
    div = np.exp(np.arange(0, H, 2, dtype=np.float32)
                 * np.float32(-math.log(10000.0) / H))
    ang = pos * div[None, :]
    te = np.stack([np.sin(ang), np.cos(ang)], axis=-1).reshape(S, H)
    te = te.astype(np.float32)
    idx = np.arange(S)
    conn = (1.0 / (1.0 + np.abs(idx[:, None] - idx[None, :]).astype(np.float32)))
    C = (0.1 * conn).astype(np.float32)
    return te, C


def _f32(x):
    return np.ascontiguousarray(np.asarray(x, dtype=np.float32))


def build_in_maps(timestamps, features, amplitudes,
                  We1, be1, We2, be2, g_ev, b_ev,
                  Wt1, bt1, Wt2, bt2, Wc, bc, g_sr, b_sr,
                  Wq, bq, Wk, bk, Wv, bv):
    timestamps = _f32(timestamps)
    order = np.argsort(timestamps, kind="stable")
    ts_s = timestamps[order]
    featsT = np.ascontiguousarray(_f32(features)[order].T)  # [D_IN, S]
    amps_s = _f32(amplitudes)[order]
    te, C = _host_constants()
    g_ev, b_ev = _f32(g_ev), _f32(b_ev)
    g_sr, b_sr = _f32(g_sr), _f32(b_sr)
    Wc = _f32(Wc)
    # exact algebra folds (see module docstring)
    Wc_eff = np.ascontiguousarray(np.concatenate(
        [g_ev[:, None] * Wc[:H], Wc[H:H + H // 2] + Wc[H + H // 2:]], axis=0))
    bc_eff = (_f32(bc).astype(np.float64)
              + b_ev.astype(np.float64) @ Wc[:H].astype(np.float64)
              ).astype(np.float32)
    te_b = (te.astype(np.float64) + b_sr.astype(np.float64)[None, :])
    Wg = {p: g_sr[:, None] * _f32(W) for p, W in
          (("q", Wq), ("k", Wk), ("v", Wv))}
    # te/b_sr enter ctx AFTER the g_sr gain, so they multiply the ORIGINAL W
    teW = {p: (te_b @ _f32(W).astype(np.float64)
               + _f32(b).astype(np.float64)[None, :]).astype(np.float32)
           for p, W, b in (("q", Wq, bq), ("k", Wk, bk), ("v", Wv, bv))}

    def cols(v, t):  # [t*128] -> [128, t]
        return _f32(v).reshape(t, 128).T
    pp = np.concatenate([cols(be1, 2), cols(bt1, 1), cols(bt2, 2)], axis=1)
    row = np.concatenate([_f32(Wt1).reshape(-1), bc_eff, _f32(be2)])[None, :]

    common = {
        "We1": _f32(We1), "We2": _f32(We2), "Wt2": _f32(Wt2),
        "Wc_eff": Wc_eff,
        "Wq_g": _f32(Wg["q"]), "Wk_g": _f32(Wg["k"]), "Wv_g": _f32(Wg["v"]),
        "pp_pack": np.ascontiguousarray(pp),
        "row_pack": np.ascontiguousarray(row),
    }
    in_maps = []
    for c in range(N_CORES):
        sh = slice(c * J_SH, (c + 1) * J_SH)
        base = C[:, sh]                       # [i=512, jl=64]
        conn2 = np.ascontiguousarray(
            base.reshape(NBLK, 2, J_SH).transpose(1, 2, 0).reshape(128, NBLK))
        m = dict(common)
        m["featsT_sh"] = np.ascontiguousarray(featsT[:, sh])
        m["ts_sh"] = np.ascontiguousarray(ts_s[sh])
        m["amps_sh"] = np.ascontiguousarray(amps_s[sh])
        m["teWq_sh"] = np.ascontiguousarray(teW["q"][sh])
        m["teWk_sh"] = np.ascontiguousarray(teW["k"][sh])
        m["teWv_sh"] = np.ascontiguousarray(teW["v"][sh])
        m["conn2_sh"] = conn2
        in_maps.append(m)
    return in_maps


def assemble_outputs(outs):
    K = np.concatenate([outs[c]["K_sh"] for c in range(N_CORES)], axis=1)
    Q = np.concatenate([outs[c]["q_out"] for c in range(N_CORES)],
                       axis=0).reshape(1, S, H)
    V = np.concatenate([outs[c]["v_out"] for c in range(N_CORES)],
                       axis=0).reshape(1, S, H)
    return Q, K, V


def kernel(timestamps, features, amplitudes, neuron_ids,
           We1, be1, We2, be2, g_ev, b_ev,
           Wt1, bt1, Wt2, bt2, Wc, bc, g_sr, b_sr,
           Wq, bq, Wk, bk, Wv, bv, **_unused):
    del neuron_ids  # unused by the reference model
    nc = _get_nc()
    in_maps = build_in_maps(
        timestamps, features, amplitudes,
        We1, be1, We2, be2, g_ev, b_ev,
        Wt1, bt1, Wt2, bt2, Wc, bc, g_sr, b_sr,
        Wq, bq, Wk, bk, Wv, bv)
    res = run_bass_kernel_spmd(nc, in_maps, core_ids=list(range(N_CORES)))
    return assemble_outputs(res.results)


# revision 25
# speedup vs baseline: 1.3927x; 1.0455x over previous
"""Trainium2 Bass kernel for nn_NeurosynapticEventEncoder.

Reference model:
    sort events by timestamp -> event MLP + LN -> temporal MLP ->
    concat/amp-gate -> proj 2H->H + LN -> + sinusoidal pos enc -> ctx
    Q = ctx@Wq+bq, Kb = ctx@Wk+bk, V = ctx@Wv+bv,
    K[i,j,h] = Kb[j,h] + 0.1*conn[i,j]  (conn = 1/(1+|i-j|))

Sharding: K [S,S,H] (536 MB fp32) is sharded over its SECOND axis (j)
across 8 cores.  Everything upstream of K is row-wise in s (LayerNorms
are per-row, matmuls row-independent), so each core computes only its
64-row slice of the whole chain — ctx/Kb/Q/V shards — no collectives,
no redundant compute.  Host concatenates Q/V (axis 0) and K (axis 1).

Device pipeline (per core, s-shard n=64):
  MLP1 transposed ([D, s], weights-as-stored lhsT) -> h1T
  MLP2 flipped (lhsT=h1T) -> h2 [s, H]  -> LN1 via bn_stats -> c1n
  PE-transpose c1n -> c1T; temporal MLP transposed -> t2T
  comb^T @ Wc' (+bc' bias row) -> amp-gate on the PSUM->SBUF copy
  LN2 via bn_stats -> xn [s, H] -> PE-transpose -> xnT
  Kb/Q/V = xn @ W'_p + teW_p  (teW accumulated into PSUM by an
  identity-weight matmul)
  K expansion: K_flat[(i,jl),h] = kb[jl,h] + 0.1conn[i,j0+jl] as
  [128,512] per-partition-scalar adds (DVE/ACT), 8 MB coalesced DMAs.

Exact host-side algebra folds (weight preprocessing only):
  Wc' = [g_ev*Wc[:H]; Wc[H:H+H/2]+Wc[H+H/2:]]   (t is tiled twice)
  bc' = bc + b_ev@Wc[:H]
  W'_p = g_sr*W_p;  teW_p = (te[sh]+b_sr)@W'_p + b_p  (p in {q,k,v})
"""

import math
from contextlib import ExitStack

import numpy as np

import concourse.bass as bass
import concourse.tile as tile
from concourse import bacc, mybir
from concourse.bass_utils import run_bass_kernel_spmd
from concourse.masks import make_identity

S = 512
D_IN = 256
H = 512
N_CORES = 8
J_SH = S // N_CORES       # 64 columns of K / rows of ctx per core
NBLK = S * J_SH // 128    # 256 slabs of [128, 512] per core
F32 = mybir.dt.float32
AF = mybir.ActivationFunctionType
ALU = mybir.AluOpType
EPS = 1e-5
# K-write tile sizes in slabs: small first tiles to start DMA early
KW_TILES = [8, 8, 8, 8] + [32] * 7


def _build_nc():
    nc = bacc.Bacc()
    n = J_SH

    def inp(name, shape):
        return nc.declare_dram_parameter(name, list(shape), F32, isOutput=False)

    # per-core shards
    featsT_d = inp("featsT_sh", (D_IN, n))
    ts_d = inp("ts_sh", (n,))
    amps_d = inp("amps_sh", (n,))
    teWq_d = inp("teWq_sh", (n, H))
    teWk_d = inp("teWk_sh", (n, H))
    teWv_d = inp("teWv_sh", (n, H))
    C_d = inp("conn2_sh", (128, NBLK))
    # replicated (pre-folded) weights
    We1_d = inp("We1", (D_IN, H // 2))
    We2_d = inp("We2", (H // 2, H))
    Wt2_d = inp("Wt2", (H // 4, H // 2))
    Wc_d = inp("Wc_eff", (H + H // 2, H))
    Wq_d = inp("Wq_g", (H, H))
    Wk_d = inp("Wk_g", (H, H))
    Wv_d = inp("Wv_g", (H, H))
    pp_d = inp("pp_pack", (128, 5))      # be1(2) bt1(1) bt2(2)
    row_d = inp("row_pack", (1, H // 4 + 2 * H))  # Wt1 | bc' | be2

    Ksh_d = nc.declare_dram_parameter("K_sh", [S, J_SH, H], F32, isOutput=True)
    q_d = nc.declare_dram_parameter("q_out", [n, H], F32, isOutput=True)
    v_d = nc.declare_dram_parameter("v_out", [n, H], F32, isOutput=True)

    with tile.TileContext(nc) as tc:
        with ExitStack() as ctx:
            persist = ctx.enter_context(tc.tile_pool(name="persist", bufs=1))
            ps_mm = ctx.enter_context(
                tc.tile_pool(name="ps_mm", bufs=2, space=bass.MemorySpace.PSUM))
            ps_sm = ctx.enter_context(
                tc.tile_pool(name="ps_sm", bufs=2, space=bass.MemorySpace.PSUM))

            ones_row = persist.tile([1, 128], F32)
            nc.vector.memset(ones_row, 1.0)
            eps_sb = persist.tile([128, 1], F32)
            nc.vector.memset(eps_sb, EPS)
            ident = persist.tile([128, 128], F32)
            make_identity(nc, ident)
            kb2_sb = persist.tile([128, H], F32)
            C_sb = persist.tile([128, NBLK], F32)

            # PE-HAM warmup: ~5us of dependency-free matmuls into a scratch
            # PSUM bank while input DMAs are in flight, so the real phase-1
            # matmuls start at the 2.4 GHz warm clock instead of 1.2 GHz.
            ps_w = ctx.enter_context(
                tc.tile_pool(name="ps_w", bufs=1, space=bass.MemorySpace.PSUM))
            warm_sb = persist.tile([128, H], F32)
            nc.vector.memset(warm_sb, 0.0)
            warm_ps = ps_w.tile([128, H], F32)
            for _ in range(12):
                nc.tensor.matmul(warm_ps, ident, warm_sb,
                                 start=True, stop=True)

            with tc.tile_pool(name="p1", bufs=1) as p1:
                def load(name, shape, dram_ap, engine=None):
                    t = p1.tile(shape, F32, name=name)
                    (engine or nc.sync).dma_start(out=t, in_=dram_ap)
                    return t

                feats_sb = load("feats_sb", [128, 2, n],
                                featsT_d[:].rearrange("(kt p) s -> p kt s", p=128))
                We1_sb = load("We1_sb", [128, 2, H // 2],
                              We1_d[:].rearrange("(kt p) m -> p kt m", p=128))
                pp_sb = load("pp_sb", [128, 5], pp_d[:], nc.gpsimd)
                be1_sb = pp_sb[:, 0:2]
                bt1_sb = pp_sb[:, 2:3]
                bt2_sb = pp_sb[:, 3:5]
                row_sb = load("row_sb", [1, H // 4 + 2 * H], row_d[:])
                Wt1_sb = row_sb[:, 0:128]
                bc_sb = row_sb[:, 128:640]
                be2_sb = row_sb[:, 640:1152]
                ts_sb = load("ts_sb", [1, n], ts_d[:].unsqueeze(0), nc.gpsimd)
                amps_sb = load("amps_sb", [n, 1], amps_d[:].unsqueeze(-1),
                               nc.gpsimd)
                amps_row = load("amps_row", [1, n], amps_d[:].unsqueeze(0),
                                nc.gpsimd)
                We2_sb = load("We2_sb", [128, 2, H],
                              We2_d[:].rearrange("(kt p) m -> p kt m", p=128))
                Wt2_sb = load("Wt2_sb", [128, H // 2], Wt2_d[:])
                Wc_sb = load("Wc_sb", [128, 6, H],
                             Wc_d[:].rearrange("(kt p) m -> p kt m", p=128))
                Wk_sb = load("Wk_sb", [128, 4, H],
                             Wk_d[:].rearrange("(kt p) m -> p kt m", p=128))
                teWk_sb = load("teWk_sb", [n, H], teWk_d[:])
                Wq_sb = load("Wq_sb", [128, 4, H],
                             Wq_d[:].rearrange("(kt p) m -> p kt m", p=128))
                teWq_sb = load("teWq_sb", [n, H], teWq_d[:])
                Wv_sb = load("Wv_sb", [128, 4, H],
                             Wv_d[:].rearrange("(kt p) m -> p kt m", p=128))
                teWv_sb = load("teWv_sb", [n, H], teWv_d[:])
                nc.sync.dma_start(out=C_sb, in_=C_d[:])

                # ---- event MLP layer 1 (transposed): h1T [H/2, s] ----
                h1_sb = p1.tile([128, 2, n], F32)
                for mt in range(2):
                    mm = ps_sm.tile([128, n], F32, name="mm_sm")
                    for kt in range(2):
                        nc.tensor.matmul(
                            mm, We1_sb[:, kt, bass.ts(mt, 128)],
                            feats_sb[:, kt, :], start=(kt == 0), stop=(kt == 1))
                    nc.scalar.activation(h1_sb[:, mt, :], mm, AF.Relu,
                                         bias=be1_sb[:, mt:mt + 1], scale=1.0)

                # ---- temporal MLP (transposed): t2T [H/2, s] ----
                t1_ps = ps_sm.tile([128, n], F32, name="mm_sm")
                nc.tensor.matmul(t1_ps, Wt1_sb, ts_sb, start=True, stop=True)
                t1_sb = p1.tile([128, n], F32)
                nc.scalar.activation(t1_sb, t1_ps, AF.Relu,
                                     bias=bt1_sb[:, 0:1], scale=1.0)
                t2_sb = p1.tile([128, 2, n], F32)
                for mt in range(2):
                    mm = ps_sm.tile([128, n], F32, name="mm_sm")
                    nc.tensor.matmul(mm, Wt2_sb[:, bass.ts(mt, 128)], t1_sb,
                                     start=True, stop=True)
                    nc.scalar.activation(t2_sb[:, mt, :], mm, AF.Identity,
                                         bias=bt2_sb[:, mt:mt + 1], scale=1.0)

                # ---- event MLP layer 2 (flipped): h2 [s, H] ----
                h2_ps = ps_mm.tile([n, H], F32, name="mm_big")
                for kt in range(2):
                    nc.tensor.matmul(h2_ps, h1_sb[:, kt, :], We2_sb[:, kt, :],
                                     start=(kt == 0), stop=False)
                nc.tensor.matmul(h2_ps, ones_row[:, 0:n], be2_sb,
                                 start=False, stop=True)
                h2_sb = p1.tile([n, H], F32)
                nc.scalar.copy(h2_sb, h2_ps)

                # ---- LN1 (free axis, bn_stats); g/b folded into Wc'/bc' ----
                def ln_normalize(x_sb, out_sb, tagn):
                    stats = p1.tile([n, nc.vector.BN_STATS_DIM], F32,
                                    name=f"st{tagn}")
                    nc.vector.bn_stats(out=stats, in_=x_sb)
                    mv = p1.tile([n, nc.vector.BN_AGGR_DIM], F32,
                                 name=f"mv{tagn}")
                    nc.vector.bn_aggr(out=mv, in_=stats)
                    std_c = p1.tile([n, 1], F32, name=f"sd{tagn}")
                    nc.scalar.activation(std_c, mv[:, 1:2], AF.Sqrt,
                                         bias=eps_sb[0:n, 0:1], scale=1.0)
                    rstd_c = p1.tile([n, 1], F32, name=f"rs{tagn}")
                    nc.vector.reciprocal(rstd_c, std_c)
                    nc.vector.tensor_scalar(out=out_sb, in0=x_sb,
                                            scalar1=mv[:, 0:1], scalar2=rstd_c,
                                            op0=ALU.subtract, op1=ALU.mult)

                c1n_sb = p1.tile([n, H], F32)
                ln_normalize(h2_sb, c1n_sb, "1")

                # transpose c1n -> c1T [H, s]
                c1T_sb = p1.tile([128, 4, n], F32)
                for ht in range(4):
                    tp = ps_sm.tile([128, n], F32, name="mm_sm")
                    nc.tensor.transpose(tp, c1n_sb[:, bass.ts(ht, 128)],
                                        ident[0:n, 0:n])
                    nc.scalar.copy(c1T_sb[:, ht, :], tp)

                # ---- comb^T @ Wc'; +bc'/amps via inv-amps outer product so
                # the amp-gate on the copy restores an UNgated bc' ----
                ia_sb = p1.tile([1, n], F32)
                nc.vector.reciprocal(ia_sb, amps_row)
                comb_kt = [c1T_sb[:, 0, :], c1T_sb[:, 1, :], c1T_sb[:, 2, :],
                           c1T_sb[:, 3, :], t2_sb[:, 0, :], t2_sb[:, 1, :]]
                post_ps = ps_mm.tile([n, H], F32, name="mm_big")
                for kt in range(6):
                    nc.tensor.matmul(post_ps, comb_kt[kt], Wc_sb[:, kt, :],
                                     start=(kt == 0), stop=False)
                nc.tensor.matmul(post_ps, ia_sb, bc_sb,
                                 start=False, stop=True)
                post_sb = p1.tile([n, H], F32)
                nc.scalar.activation(post_sb, post_ps, AF.Copy,
                                     bias=0.0, scale=amps_sb)

                # ---- LN2 -> xn [s, H]; transpose -> xnT ----
                xn_sb = p1.tile([n, H], F32)
                ln_normalize(post_sb, xn_sb, "2")
                xnT_sb = p1.tile([128, 4, n], F32)
                for ht in range(4):
                    tp = ps_sm.tile([128, n], F32, name="mm_sm")
                    nc.tensor.transpose(tp, xn_sb[:, bass.ts(ht, 128)],
                                        ident[0:n, 0:n])
                    nc.scalar.copy(xnT_sb[:, ht, :], tp)

                # ---- projections: out = xn @ W'_p + teW_p ----
                def project(W_sb, teW_sb, out_sb):
                    mm = ps_mm.tile([n, H], F32, name="mm_big")
                    for kt in range(4):
                        nc.tensor.matmul(mm, xnT_sb[:, kt, :], W_sb[:, kt, :],
                                         start=(kt == 0), stop=False)
                    nc.tensor.matmul(mm, ident[0:n, 0:n], teW_sb,
                                     start=False, stop=True)
                    nc.scalar.copy(out_sb, mm)

                project(Wk_sb, teWk_sb, kb2_sb[0:n, :])
                nc.gpsimd.dma_start(out=kb2_sb[n:2 * n, :], in_=kb2_sb[0:n, :])

                q_sb = p1.tile([n, H], F32)
                project(Wq_sb, teWq_sb, q_sb)
                nc.sync.dma_start(out=q_d[:], in_=q_sb)
                v_sb = p1.tile([n, H], F32)
                project(Wv_sb, teWv_sb, v_sb)
                nc.sync.dma_start(out=v_d[:], in_=v_sb)

            # ---- K expansion ----
            # K_sh[i, jl, h]; flat row i*64+jl; partition p=(i%2)*64+jl.
            kw = ctx.enter_context(tc.tile_pool(name="kw", bufs=2))
            Kv = Ksh_d[:].rearrange("(blk a) j h -> (a j) blk h", a=2)
            nslab = 0
            b0 = 0
            for tsize in KW_TILES:
                kT = kw.tile([128, tsize, H], F32,
                             name=f"kT{tsize}", tag=f"kT{tsize}")
                for m in range(tsize):
                    blk = b0 + m
                    slab = kT[:, m, :]
                    if nslab % 5 < 3:
                        nc.vector.tensor_scalar_add(slab, kb2_sb,
                                                    C_sb[:, blk:blk + 1])
                    else:
                        nc.scalar.add(slab, kb2_sb, C_sb[:, blk:blk + 1])
                    nslab += 1
                nc.sync.dma_start(out=Kv[:, b0:b0 + tsize, :], in_=kT)
                b0 += tsize
            assert b0 == NBLK

    nc.finalize()  # Bacc: wait-splitting + register allocation passes
    return nc


_NC = None


def _get_nc():
    global _NC
    if _NC is None:
        _NC = _build_nc()
    return _NC


# ----------------------------------------------------------------------------
# host side
# ----------------------------------------------------------------------------

def _host_constants():
    pos = np.arange(S, dtype=np.float32)[:, None]
    div = np.exp(np.arange(0, H, 2, dtype=np.float32)
                 * np.float32(-math.log(10000.0) / H))
    ang = pos * div[None, :]
    te = np.stack([np.sin(ang), np.cos(ang)], axis=-1).reshape(S, H)
    te = te.astype(np.float32)
    idx = np.arange(S)
    conn = (1.0 / (1.0 + np.abs(idx[:, None] - idx[None, :]).astype(np.float32)))
    C = (0.1 * conn).astype(np.float32)
    return te, C


def _f32(x):
    return np.ascontiguousarray(np.asarray(x, dtype=np.float32))


def build_in_maps(timestamps, features, amplitudes,
                  We1, be1, We2, be2, g_ev, b_ev,
                  Wt1, bt1, Wt2, bt2, Wc, bc, g_sr, b_sr,
                  Wq, bq, Wk, bk, Wv, bv):
    timestamps = _f32(timestamps)
    order = np.argsort(timestamps, kind="stable")
    ts_s = timestamps[order]
    featsT = np.ascontiguousarray(_f32(features)[order].T)  # [D_IN, S]
    amps_s = _f32(amplitudes)[order]
    te, C = _host_constants()
    g_ev, b_ev = _f32(g_ev), _f32(b_ev)
    g_sr, b_sr = _f32(g_sr), _f32(b_sr)
    Wc = _f32(Wc)
    # exact algebra folds (see module docstring)
    Wc_eff = np.ascontiguousarray(np.concatenate(
        [g_ev[:, None] * Wc[:H], Wc[H:H + H // 2] + Wc[H + H // 2:]], axis=0))
    bc_eff = (_f32(bc).astype(np.float64)
              + b_ev.astype(np.float64) @ Wc[:H].astype(np.float64)
              ).astype(np.float32)
    te_b = (te.astype(np.float64) + b_sr.astype(np.float64)[None, :])
    Wg = {p: g_sr[:, None] * _f32(W) for p, W in
          (("q", Wq), ("k", Wk), ("v", Wv))}
    # te/b_sr enter ctx AFTER the g_sr gain, so they multiply the ORIGINAL W
    teW = {p: (te_b @ _f32(W).astype(np.float64)
               + _f32(b).astype(np.float64)[None, :]).astype(np.float32)
           for p, W, b in (("q", Wq, bq), ("k", Wk, bk), ("v", Wv, bv))}

    def cols(v, t):  # [t*128] -> [128, t]
        return _f32(v).reshape(t, 128).T
    pp = np.concatenate([cols(be1, 2), cols(bt1, 1), cols(bt2, 2)], axis=1)
    row = np.concatenate([_f32(Wt1).reshape(-1), bc_eff, _f32(be2)])[None, :]

    common = {
        "We1": _f32(We1), "We2": _f32(We2), "Wt2": _f32(Wt2),
        "Wc_eff": Wc_eff,
        "Wq_g": _f32(Wg["q"]), "Wk_g": _f32(Wg["k"]), "Wv_g": _f32(Wg["v"]),
        "pp_pack": np.ascontiguousarray(pp),
        "row_pack": np.ascontiguousarray(row),
    }
    in_maps = []
    for c in range(N_CORES):
        sh = slice(c * J_SH, (c + 1) * J_SH)
        base = C[:, sh]                       # [i=512, jl=64]
        conn2 = np.ascontiguousarray(
            base.reshape(NBLK, 2, J_SH).transpose(1, 2, 0).reshape(128, NBLK))
        m = dict(common)
        m["featsT_sh"] = np.ascontiguousarray(featsT[:, sh])
        m["ts_sh"] = np.ascontiguousarray(ts_s[sh])
        m["amps_sh"] = np.ascontiguousarray(amps_s[sh])
        m["teWq_sh"] = np.ascontiguousarray(teW["q"][sh])
        m["teWk_sh"] = np.ascontiguousarray(teW["k"][sh])
        m["teWv_sh"] = np.ascontiguousarray(teW["v"][sh])
        m["conn2_sh"] = conn2
        in_maps.append(m)
    return in_maps


def assemble_outputs(outs):
    K = np.concatenate([outs[c]["K_sh"] for c in range(N_CORES)], axis=1)
    Q = np.concatenate([outs[c]["q_out"] for c in range(N_CORES)],
                       axis=0).reshape(1, S, H)
    V = np.concatenate([outs[c]["v_out"] for c in range(N_CORES)],
                       axis=0).reshape(1, S, H)
    return Q, K, V


def kernel(timestamps, features, amplitudes, neuron_ids,
           We1, be1, We2, be2, g_ev, b_ev,
           Wt1, bt1, Wt2, bt2, Wc, bc, g_sr, b_sr,
           Wq, bq, Wk, bk, Wv, bv, **_unused):
    del neuron_ids  # unused by the reference model
    nc = _get_nc()
    in_maps = build_in_maps(
        timestamps, features, amplitudes,
        We1, be1, We2, be2, g_ev, b_ev,
        Wt1, bt1, Wt2, bt2, Wc, bc, g_sr, b_sr,
        Wq, bq, Wk, bk, Wv, bv)
    res = run_bass_kernel_spmd(nc, in_maps, core_ids=list(range(N_CORES)))
    return assemble_outputs(res.results)
